# revision 1
# baseline (speedup 1.0000x reference)
"""Trainium2 Bass kernel for nn_Experts (grouped MoE expert MLP).

Computes, for each of 8 experts e:
    h   = x_e @ w0_e.T          # [2048,1024] @ [1024,4096] -> [2048,4096]
    g   = gelu_exact(h)
    out = g @ w3_e.T            # [2048,4096] @ [4096,1024] -> [2048,1024]
then masks unpopular experts with zero gating activity (output_tensor).

Sharding: expert-parallel, 1 expert per NeuronCore across 8 cores (SPMD —
one compiled NEFF, per-core input data).

Layout strategy: all operands are pre-transposed on the host into
contraction-major ("K-major") layouts so the device kernel needs no
transposes at all:
    xT  [128, 8, 2048]  (d%128, d//128, t)   bf16
    w0T [128, 8, 4096]  (d%128, d//128, f)   bf16
    w3T [128, 32, 1024] (f%128, f//128, d)   bf16
GEMM1 produces hT tiles [f=128, t] in PSUM, GELU moves them to SBUF as bf16,
and those tiles are directly the lhsT operand of GEMM2 (contraction over f),
whose PSUM output [t=128, d] accumulates over all 32 f-chunks and lands in
the natural [t, d] layout of the output.
"""

import numpy as np
import ml_dtypes

T = 2048      # tokens (capacity) per expert
D = 1024      # hidden
F = 4096      # ffn
P = 128       # partitions
TB = 256      # token block (GEMM1 moving free dim)
NTB = T // TB
DC = D // P   # 8 d-chunks (GEMM1 contraction)
FC = F // P   # 32 f-chunks (GEMM2 contraction)
DW = 512      # GEMM2 output free-dim chunk
NUM_LOCAL = 4
N_CORES = 8

_cache = {}


def _build_nc(
    tb_size=TB,          # token block
    x_split=1,           # extra splits of each x d-chunk DMA (along t)
    w0_split=1,          # extra splits of each w0 d-chunk DMA (along f)
    w3_group=1,          # f-chunks per w3 DMA
    g_bufs=4,
    h_bufs=2,
    o_sb_bufs=4,
    dma_scheme="tuned",  # "simple" | "tuned" (critical-prefix-first ordering)
    fcg=4,               # fc per w0/w3 DMA group in tuned scheme
    pipeline_o=True,     # issue GEMM2(fc) after GEMM1(fc+1) to hide gelu latency
):
    import sys
    if "/opt/trn_rl_repo" not in sys.path:
        sys.path.insert(0, "/opt/trn_rl_repo")
    import concourse.bass as bass
    import concourse.tile as tile
    import concourse.mybir as mybir
    from concourse import bacc

    bf16 = mybir.dt.bfloat16
    f32 = mybir.dt.float32
    AFT = mybir.ActivationFunctionType

    TBS = tb_size
    NTBS = T // TBS
    NTS = TBS // P       # t-subchunks per block (GEMM2 lhsT count)
    n_ops = NTS * 2      # out psum tiles per block ([t 128] x [d 512])

    nc = bacc.Bacc(
        "TRN2",
        target_bir_lowering=False,
        debug=False,
        enable_asserts=True,
        num_devices=N_CORES,
    )

    xT = nc.dram_tensor("xT", [P, DC, T], bf16, kind="ExternalInput").ap()
    w0T = nc.dram_tensor("w0T", [P, DC, F], bf16, kind="ExternalInput").ap()
    w3T = nc.dram_tensor("w3T", [P, FC, D], bf16, kind="ExternalInput").ap()
    out = nc.dram_tensor("out", [T, D], f32, kind="ExternalOutput").ap()

    with tile.TileContext(nc) as tc:
        with (
            tc.tile_pool(name="weights", bufs=1) as wpool,
            tc.tile_pool(name="gelu", bufs=g_bufs) as gpool,
            tc.tile_pool(name="ostage", bufs=o_sb_bufs) as opool,
            tc.tile_pool(name="hps", bufs=h_bufs, space="PSUM") as hpsum,
            tc.tile_pool(name="ops", bufs=n_ops, space="PSUM") as opsum,
        ):
            x_sb = wpool.tile([P, DC, T], bf16, name="x_sb", tag="x_sb")
            w0_sb = wpool.tile([P, DC, F], bf16, name="w0_sb", tag="w0_sb")
            w3_sb = wpool.tile([P, FC, D], bf16, name="w3_sb", tag="w3_sb")

            if dma_scheme == "simple":
                # Load x and w0 first (first h-tile needs ALL d-chunks of
                # both); w3 f-chunks stream in behind.
                for dc in range(DC):
                    for s in range(x_split):
                        w = T // x_split
                        nc.sync.dma_start(x_sb[:, dc, s * w:(s + 1) * w],
                                          xT[:, dc, s * w:(s + 1) * w])
                    for s in range(w0_split):
                        w = F // w0_split
                        nc.sync.dma_start(w0_sb[:, dc, s * w:(s + 1) * w],
                                          w0T[:, dc, s * w:(s + 1) * w])
                for g in range(FC // w3_group):
                    lo, hi = g * w3_group, (g + 1) * w3_group
                    nc.sync.dma_start(w3_sb[:, lo:hi], w3T[:, lo:hi])
            else:
                # Critical-prefix-first: x for tb0, then per-f-group w0 (all
                # d-chunks) and w3 interleaved in the order GEMM1/GEMM2
                # consume them, then the rest of x.
                for dc in range(DC):
                    nc.sync.dma_start(x_sb[:, dc, 0:TBS], xT[:, dc, 0:TBS])
                for g in range(FC // fcg):
                    flo, fhi = g * fcg * P, (g + 1) * fcg * P
                    for dc in range(DC):
                        nc.sync.dma_start(w0_sb[:, dc, flo:fhi],
                                          w0T[:, dc, flo:fhi])
                    nc.sync.dma_start(w3_sb[:, g * fcg:(g + 1) * fcg],
                                      w3T[:, g * fcg:(g + 1) * fcg])
                for tb in range(1, NTBS):
                    for dc in range(DC):
                        nc.sync.dma_start(
                            x_sb[:, dc, tb * TBS:(tb + 1) * TBS],
                            xT[:, dc, tb * TBS:(tb + 1) * TBS])

            for tb in range(NTBS):
                o_ps = [
                    opsum.tile([P, DW], f32, name=f"o_ps_{tb}_{i}", tag="o_ps")
                    for i in range(n_ops)
                ]

                def emit_o(fc, g_sb):
                    for ts in range(NTS):
                        for dc2 in range(2):
                            nc.tensor.matmul(
                                o_ps[ts * 2 + dc2][:],
                                g_sb[:, ts * P:(ts + 1) * P],
                                w3_sb[:, fc, dc2 * DW:(dc2 + 1) * DW],
                                start=(fc == 0),
                                stop=(fc == FC - 1),
                            )

                pending = None
                for fc in range(FC):
                    h_ps = hpsum.tile([P, TBS], f32, name=f"h_ps_{tb}_{fc}", tag="h_ps")
                    for dc in range(DC):
                        nc.tensor.matmul(
                            h_ps[:],
                            w0_sb[:, dc, fc * P:(fc + 1) * P],
                            x_sb[:, dc, tb * TBS:(tb + 1) * TBS],
                            start=(dc == 0),
                            stop=(dc == DC - 1),
                        )
                    g_sb = gpool.tile([P, TBS], bf16, name=f"g_{tb}_{fc}", tag="g")
                    nc.scalar.activation(g_sb[:], h_ps[:], AFT.Gelu)
                    if not pipeline_o:
                        emit_o(fc, g_sb)
                    else:
                        if pending is not None:
                            emit_o(*pending)
                        pending = (fc, g_sb)
                if pending is not None:
                    emit_o(*pending)

                for ts in range(NTS):
                    for dc2 in range(2):
                        o_sb = opool.tile([P, DW], f32, name=f"o_sb_{tb}_{ts}_{dc2}",
                                          tag="o_sb")
                        nc.vector.tensor_copy(o_sb[:], o_ps[ts * 2 + dc2][:])
                        nc.sync.dma_start(
                            out[tb * TBS + ts * P: tb * TBS + (ts + 1) * P,
                                dc2 * DW:(dc2 + 1) * DW],
                            o_sb[:],
                        )

    nc.compile()
    return nc


def _build_nc_v2(
    g_extra=0,           # extra gelu-tile slots beyond FC (lookahead into next block)
    h_bufs=3,
    o_ps_bufs=2,
    o_sb_bufs=3,
    x_bufs=2,
    fcg=4,               # fc per w0/w3 DMA group
    x_coarse=True,       # one DMA per x block vs per-dc
    w0_coarse=False,     # one DMA per w0 f-group vs per-dc
    warmup_mms=8,        # scratch matmuls issued before the real work so the
                         # PE rides out the HAM cold-clock window during the
                         # initial DMA wait instead of during real matmuls
):
    """TB=512 two-phase variant: per 512-token block, phase A runs GEMM1+GELU
    for all 32 f-chunks (g tiles [128,512] bf16 stay in SBUF), phase B runs
    GEMM2 as 8 sequential PSUM accumulation groups (one [t=128, d=512] output
    tile each, contraction over all 32 f-chunks). x is streamed per-block
    instead of fully resident to stay under the SBUF cap."""
    import sys
    if "/opt/trn_rl_repo" not in sys.path:
        sys.path.insert(0, "/opt/trn_rl_repo")
    import concourse.tile as tile
    import concourse.mybir as mybir
    from concourse import bacc

    bf16 = mybir.dt.bfloat16
    f32 = mybir.dt.float32
    AFT = mybir.ActivationFunctionType

    TBS = 512
    NTBS = T // TBS      # 4
    NTS = TBS // P       # 4

    G = FC // fcg        # w0 DMA groups
    FW = fcg * P         # f elements per group (512)

    nc = bacc.Bacc(
        "TRN2",
        target_bir_lowering=False,
        debug=False,
        enable_asserts=True,
        num_devices=N_CORES,
    )

    # DRAM layouts are grouped so every load has long (8KB) contiguous
    # per-partition runs: xT by token-block, w0T by f-group.
    xT = nc.dram_tensor("xT", [P, NTBS, DC, TBS], bf16, kind="ExternalInput").ap()
    w0T = nc.dram_tensor("w0T", [P, G, DC, FW], bf16, kind="ExternalInput").ap()
    w3T = nc.dram_tensor("w3T", [P, FC, D], bf16, kind="ExternalInput").ap()
    out = nc.dram_tensor("out", [T, D], f32, kind="ExternalOutput").ap()

    with tile.TileContext(nc) as tc:
        with (
            tc.tile_pool(name="weights", bufs=1) as wpool,
            tc.tile_pool(name="xin", bufs=x_bufs) as xpool,
            tc.tile_pool(name="gelu", bufs=FC + g_extra) as gpool,
            tc.tile_pool(name="ostage", bufs=o_sb_bufs) as opool,
            tc.tile_pool(name="hps", bufs=h_bufs, space="PSUM") as hpsum,
            tc.tile_pool(name="ops", bufs=o_ps_bufs, space="PSUM") as opsum,
        ):
            # w0 SBUF mirrors the grouped DRAM layout; GEMM1 slices
            # [:, fc//fcg, dc, (fc%fcg)*P : +P].
            w0_sb = wpool.tile([P, G, DC, FW], bf16, name="w0_sb", tag="w0_sb")
            w3_sb = wpool.tile([P, FC, D], bf16, name="w3_sb", tag="w3_sb")

            x_tiles = {}
            def load_x(tb):
                xt = xpool.tile([P, DC, TBS], bf16, name=f"x_{tb}", tag="x")
                if x_coarse:
                    nc.sync.dma_start(xt[:], xT[:, tb])
                else:
                    for dc in range(DC):
                        nc.sync.dma_start(xt[:, dc], xT[:, tb, dc])
                x_tiles[tb] = xt

            if warmup_mms:
                with (
                    tc.tile_pool(name="warm", bufs=1) as warmpool,
                    tc.tile_pool(name="warmps", bufs=1, space="PSUM") as warmpsum,
                ):
                    wsrc = warmpool.tile([P, DW], bf16, name="wsrc", tag="wsrc")
                    wps = warmpsum.tile([P, DW], f32, name="wps", tag="wps")
                    nc.gpsimd.memset(wsrc[:], 0.0)
                    for i in range(warmup_mms):
                        nc.tensor.matmul(wps[:], wsrc[:, :P], wsrc[:],
                                         start=(i == 0), stop=(i == warmup_mms - 1))

            # critical prefix: x[tb0], then w0/w3 by f-group in consumption order
            load_x(0)
            for g in range(G):
                if w0_coarse:
                    nc.sync.dma_start(w0_sb[:, g], w0T[:, g])
                else:
                    for dc in range(DC):
                        nc.sync.dma_start(w0_sb[:, g, dc], w0T[:, g, dc])
                nc.sync.dma_start(w3_sb[:, g * fcg:(g + 1) * fcg],
                                  w3T[:, g * fcg:(g + 1) * fcg])

            for tb in range(NTBS):
                if tb + 1 < NTBS:
                    load_x(tb + 1)
                xt = x_tiles.pop(tb)
                # phase A: GEMM1 + GELU for all fc
                g_tiles = []
                for fc in range(FC):
                    h_ps = hpsum.tile([P, TBS], f32, name=f"h_{tb}_{fc}", tag="h_ps")
                    for dc in range(DC):
                        j = fc % fcg
                        nc.tensor.matmul(
                            h_ps[:],
                            w0_sb[:, fc // fcg, dc, j * P:(j + 1) * P],
                            xt[:, dc],
                            start=(dc == 0),
                            stop=(dc == DC - 1),
                        )
                    g_sb = gpool.tile([P, TBS], bf16, name=f"g_{tb}_{fc}", tag="g")
                    nc.scalar.activation(g_sb[:], h_ps[:], AFT.Gelu)
                    g_tiles.append(g_sb)
                # phase B: GEMM2, one [t=128, d=512] accumulation group at a time
                for ts in range(NTS):
                    for dc2 in range(2):
                        o_ps = opsum.tile([P, DW], f32, name=f"o_{tb}_{ts}_{dc2}",
                                          tag="o_ps")
                        for fc in range(FC):
                            nc.tensor.matmul(
                                o_ps[:],
                                g_tiles[fc][:, ts * P:(ts + 1) * P],
                                w3_sb[:, fc, dc2 * DW:(dc2 + 1) * DW],
                                start=(fc == 0),
                                stop=(fc == FC - 1),
                            )
                        o_sb = opool.tile([P, DW], f32, name=f"os_{tb}_{ts}_{dc2}",
                                          tag="o_sb")
                        nc.vector.tensor_copy(o_sb[:], o_ps[:])
                        nc.sync.dma_start(
                            out[tb * TBS + ts * P: tb * TBS + (ts + 1) * P,
                                dc2 * DW:(dc2 + 1) * DW],
                            o_sb[:],
                        )

    nc.compile()
    return nc


def _get_nc():
    # v1 (_build_nc) predates the grouped DRAM layouts and is kept only for
    # reference; the host prep below feeds _build_nc_v2's layouts.
    if "nc" not in _cache:
        _cache["nc"] = _build_nc_v2()
    return _cache["nc"]


def _make_cached_fn(nc):
    """Build a reusable jitted 8-core executable around bass2jax's bass_exec
    primitive (the same lowering run_bass_kernel_spmd uses under axon), so
    repeat kernel() calls skip retrace/relower."""
    import jax
    import numpy as np
    from jax.sharding import Mesh, PartitionSpec
    try:
        from jax.experimental.shard_map import shard_map
    except ImportError:
        from jax.shard_map import shard_map
    import concourse.mybir as mybir
    from concourse.bass2jax import (_bass_exec_p, install_neuronx_cc_hook,
                                    partition_id_tensor)

    install_neuronx_cc_hook()
    partition_name = nc.partition_id_tensor.name if nc.partition_id_tensor else None
    in_names, out_names, out_avals, zero_shapes = [], [], [], []
    for alloc in nc.m.functions[0].allocations:
        if not isinstance(alloc, mybir.MemoryLocationSet):
            continue
        name = alloc.memorylocations[0].name
        if alloc.kind == "ExternalInput":
            if name != partition_name:
                in_names.append(name)
        elif alloc.kind == "ExternalOutput":
            out_names.append(name)
            shape = tuple(alloc.tensor_shape)
            dtype = mybir.dt.np(alloc.dtype)
            out_avals.append(jax.core.ShapedArray(shape, dtype))
            zero_shapes.append((shape, dtype))
    n_params = len(in_names)
    all_in_names = list(in_names) + list(out_names)
    if partition_name is not None:
        all_in_names.append(partition_name)

    def _body(*args):
        ins = list(args[:n_params])
        outs = list(args[n_params:])
        extra = [partition_id_tensor()] if partition_name is not None else []
        return tuple(_bass_exec_p.bind(
            *ins, *outs, *extra,
            out_avals=tuple(out_avals),
            in_names=tuple(all_in_names),
            out_names=tuple(out_names),
            lowering_input_output_aliases=(),
            sim_require_finite=True,
            sim_require_nnan=True,
            nc=nc,
        ))

    devices = jax.devices()[:N_CORES]
    mesh = Mesh(np.asarray(devices), ("core",))
    fn = jax.jit(
        shard_map(_body, mesh=mesh,
                  in_specs=(PartitionSpec("core"),) * (n_params + len(out_names)),
                  out_specs=(PartitionSpec("core"),) * len(out_names),
                  check_rep=False),
        keep_unused=True)

    def run(in_maps):
        concat_in = [np.concatenate([np.asarray(m[n]) for m in in_maps], axis=0)
                     for n in in_names]
        concat_zeros = [np.zeros((N_CORES * s[0], *s[1:]), dt)
                        for s, dt in zero_shapes]
        outs = fn(*concat_in, *concat_zeros)
        return [
            {name: np.asarray(outs[i]).reshape(N_CORES, *out_avals[i].shape)[c]
             for i, name in enumerate(out_names)}
            for c in range(N_CORES)
        ]

    return run


def kernel(**inputs):
    import os
    import sys
    if "/opt/trn_rl_repo" not in sys.path:
        sys.path.insert(0, "/opt/trn_rl_repo")
    from concourse import bass_utils

    output_tensor = np.asarray(inputs["output_tensor"], dtype=np.float32)  # [1, 8]
    x = np.asarray(inputs["inputs"], dtype=np.float32)   # [1, 8, 2048, 1024]
    w0 = np.asarray(inputs["w0"], dtype=np.float32)      # [8, 4096, 1024]
    w3 = np.asarray(inputs["w3"], dtype=np.float32)      # [8, 1024, 4096]

    bf = ml_dtypes.bfloat16
    TBS, NTBS, FCG = 512, T // 512, 4
    G, FW = FC // FCG, FCG * P

    def prep_expert(e):
        # cast to bf16 first (halves bytes moved by the transposes)
        xe = x[0, e].astype(bf)     # [t, d]
        w0e = w0[e].astype(bf)      # [f, d]
        w3e = w3[e].astype(bf)      # [d, f]
        # Layouts are contraction-major (partition = contraction dim % 128)
        # and grouped by DMA unit so each load is one long contiguous run per
        # partition:
        #   xT  [128, 4 tb, 8 dc, 512 t],  w0T [128, 8 g, 8 dc, 512 f],
        #   w3T [128, 32 fc, 1024 d]
        return {
            "xT": np.ascontiguousarray(
                xe.T.reshape(DC, P, NTBS, TBS).transpose(1, 2, 0, 3)),
            "w0T": np.ascontiguousarray(
                w0e.T.reshape(DC, P, G, FW).transpose(1, 2, 0, 3)),
            "w3T": np.ascontiguousarray(
                w3e.T.reshape(FC, P, D).transpose(1, 0, 2)),
        }

    from concurrent.futures import ThreadPoolExecutor
    with ThreadPoolExecutor(max_workers=N_CORES) as pool:
        in_maps = list(pool.map(prep_expert, range(N_CORES)))

    nc = _get_nc()
    results = None
    if "fast_fn" in _cache:
        try:
            results = _cache["fast_fn"](in_maps)
        except Exception:
            results = None
    if results is None:
        try:
            results = bass_utils.run_bass_kernel_spmd(
                nc, in_maps, core_ids=list(range(N_CORES))).results
        except ModuleNotFoundError:
            # trace path requested via env but axon NTFF hook missing
            os.environ["BASS_NEVER_TRACE"] = "1"
            results = bass_utils.run_bass_kernel_spmd(
                nc, in_maps, core_ids=list(range(N_CORES))).results
        try:
            fast = _make_cached_fn(nc)
            fast(in_maps)  # warm: jit trace + XLA/NEFF compile happens here
            _cache["fast_fn"] = fast
        except Exception:
            pass
    out_full = np.stack([results[e]["out"] for e in range(N_CORES)])[None]

    # unpopular experts with zero gating activity produce zeros
    unpop = output_tensor[:, NUM_LOCAL:].sum(axis=0) != 0
    mask = np.concatenate([np.ones(NUM_LOCAL, dtype=bool), unpop])
    out_full = out_full * mask[None, :, None, None].astype(np.float32)
    return out_full.astype(np.float32)



# revision 3
# speedup vs baseline: 1.1900x; 1.1900x over previous
"""Trainium2 Bass kernel for nn_Experts (grouped MoE expert MLP).

Computes, for each of 8 experts e:
    h   = x_e @ w0_e.T          # [2048,1024] @ [1024,4096] -> [2048,4096]
    g   = gelu_exact(h)
    out = g @ w3_e.T            # [2048,4096] @ [4096,1024] -> [2048,1024]
then masks unpopular experts with zero gating activity (output_tensor).

Sharding: expert-parallel, 1 expert per NeuronCore across 8 cores (SPMD —
one compiled NEFF, per-core input data).

Numerics/perf strategy: fp8 (e4m3) matmuls in DoubleRow perf mode (K=256 per
matmul, 0.5 cycles/row) with residual-corrected operands. Every GEMM is
evaluated as three fp8 term-GEMMs accumulated in ONE PSUM group:

    x @ W ~= X_hi @ W_hi + X_lo @ W_hi + X_hi @ W_lo

where X_hi = e4m3(x), X_lo = e4m3(x - X_hi) (the residual is representable
unscaled because x ~ N(0,1)), and W is pre-scaled by SW=64 on the host so
BOTH its hi part and its residual stay clear of e4m3's subnormal floor. All
three terms then share the same global scale (SW), so they can accumulate
into a single PSUM bank with no combine pass; the SW descale folds into the
GELU activation's input scale (GEMM1) or the output copy's scale (GEMM2).
Measured end-to-end rel err of this scheme: ~2.6e-3 (limit 2e-2).

g is re-quantized the same way: gelu writes G_hi = e4m3(g) and a bf16 copy
g_f; the DVE computes G_lo = e4m3(g_f - G_hi) in one scalar_tensor_tensor op.
"""

import numpy as np
import ml_dtypes

T = 2048      # tokens (capacity) per expert
D = 1024      # hidden
F = 4096      # ffn
P = 128       # partitions
TBS = 256     # token block (GEMM1 moving free dim = 2*256 DR-packed)
NTB = T // TBS        # 8
DPAIR = D // (2 * P)  # 4  k-pairs in GEMM1 contraction
FPAIR = F // (2 * P)  # 16 k-pairs in GEMM2 contraction
FC = F // P           # 32 f-chunks (GEMM1 output tiles per token block)
DW = 256              # GEMM2 output free-dim chunk
ND = D // DW          # 4
NTS = TBS // P        # 2 t-subblocks per token block
SW = 64.0             # global weight pre-scale (power of 2)
NUM_LOCAL = 4
N_CORES = 8

_cache = {}


def _build_nc_fp8(
    g_bufs=2,        # generations of the per-block G_hi/G_lo tiles
    gf_bufs=6,       # bf16 gelu scratch ring
    h_bufs=3,        # GEMM1 PSUM tiles in flight
    o_ps_bufs=4,     # GEMM2 PSUM tiles in flight
    o_sb_bufs=6,     # output staging ring
    warmup_mms=40,   # scratch matmuls riding out the PE cold-clock window
                     # while the initial DMAs land
    wgrp=8,          # fc per W0 DMA chunk
    w3grp=4,         # jj per W3 DMA chunk
):
    import sys
    if "/opt/trn_rl_repo" not in sys.path:
        sys.path.insert(0, "/opt/trn_rl_repo")
    import concourse.tile as tile
    import concourse.mybir as mybir
    from concourse import bacc

    bf16 = mybir.dt.bfloat16
    f32 = mybir.dt.float32
    e4 = mybir.dt.float8e4
    AFT = mybir.ActivationFunctionType
    PM = mybir.MatmulPerfMode
    ALU = mybir.AluOpType

    nc = bacc.Bacc(
        "TRN2",
        target_bir_lowering=False,
        debug=False,
        enable_asserts=True,
        num_devices=N_CORES,
        dynamic_dma_scratch_size=2048,
    )

    # DRAM layouts (host-prepared, all contraction-major and DMA-contiguous):
    #   w0*[p, fc, j, i, m] = W0[f=fc*128+m, d=(2j+i)*128+p]
    #   x* [p, tb, j, i, t] = X [token=tb*256+t, d=(2j+i)*128+p]
    #   w3*[p, jj, i, dd]   = W3[dd, f=(2jj+i)*128+p]
    w0h = nc.dram_tensor("w0h", [P, FC, DPAIR, 2, P], e4, kind="ExternalInput").ap()
    w0l = nc.dram_tensor("w0l", [P, FC, DPAIR, 2, P], e4, kind="ExternalInput").ap()
    xh = nc.dram_tensor("xh", [P, NTB, DPAIR, 2, TBS], e4, kind="ExternalInput").ap()
    xl = nc.dram_tensor("xl", [P, NTB, DPAIR, 2, TBS], e4, kind="ExternalInput").ap()
    w3h = nc.dram_tensor("w3h", [P, FPAIR, 2, D], e4, kind="ExternalInput").ap()
    w3l = nc.dram_tensor("w3l", [P, FPAIR, 2, D], e4, kind="ExternalInput").ap()
    out = nc.dram_tensor("out", [T, D], f32, kind="ExternalOutput").ap()

    with tile.TileContext(nc) as tc:
        with (
            tc.tile_pool(name="weights", bufs=1) as wpool,
            tc.tile_pool(name="gtiles", bufs=g_bufs) as gpool,
            tc.tile_pool(name="gf", bufs=gf_bufs) as gfpool,
            tc.tile_pool(name="ostage", bufs=o_sb_bufs) as opool,
            tc.tile_pool(name="hps", bufs=h_bufs, space="PSUM") as hpsum,
            tc.tile_pool(name="ops", bufs=o_ps_bufs, space="PSUM") as opsum,
        ):
            w0h_sb = wpool.tile([P, FC, DPAIR, 2, P], e4, name="w0h_sb", tag="w0h")
            w0l_sb = wpool.tile([P, FC, DPAIR, 2, P], e4, name="w0l_sb", tag="w0l")
            xh_sb = wpool.tile([P, NTB, DPAIR, 2, TBS], e4, name="xh_sb", tag="xh")
            xl_sb = wpool.tile([P, NTB, DPAIR, 2, TBS], e4, name="xl_sb", tag="xl")
            w3h_sb = wpool.tile([P, FPAIR, 2, D], e4, name="w3h_sb", tag="w3h")
            w3l_sb = wpool.tile([P, FPAIR, 2, D], e4, name="w3l_sb", tag="w3l")

            if warmup_mms:
                with (
                    tc.tile_pool(name="warm", bufs=1) as warmpool,
                    tc.tile_pool(name="warmps", bufs=1, space="PSUM") as warmpsum,
                ):
                    wsrc = warmpool.tile([P, 512], bf16, name="wsrc", tag="wsrc")
                    wps = warmpsum.tile([P, 512], f32, name="wps", tag="wps")
                    nc.gpsimd.memset(wsrc[:], 0.0)
                    for i in range(warmup_mms):
                        nc.tensor.matmul(wps[:], wsrc[:, :P], wsrc[:],
                                         start=(i == 0), stop=(i == warmup_mms - 1))

            # DMA issue order = consumption order: x(tb0) first, then W0/W3
            # chunks interleaved so phase A's fc-groups and phase B's jj-groups
            # arrive just ahead of the PE, then the remaining token blocks.
            nc.sync.dma_start(xh_sb[:, 0], xh[:, 0])
            nc.sync.dma_start(xl_sb[:, 0], xl[:, 0])
            w0_chunks = [(w0h_sb, w0h), (w0l_sb, w0l)]
            w3_chunks = [(w3h_sb, w3h), (w3l_sb, w3l)]
            NWG, NW3G = FC // wgrp, FPAIR // w3grp
            sched = []
            for g in range(NWG):
                sched.append(("w0", g))
                if g >= 1 and (g - 1) < NW3G:
                    sched.append(("w3", g - 1))
            for g in range(NWG - 1, NW3G):
                sched.append(("w3", g))
            for kind, g in sched:
                if kind == "w0":
                    for sb_t, dr in w0_chunks:
                        nc.sync.dma_start(sb_t[:, g * wgrp:(g + 1) * wgrp],
                                          dr[:, g * wgrp:(g + 1) * wgrp])
                else:
                    for sb_t, dr in w3_chunks:
                        nc.sync.dma_start(sb_t[:, g * w3grp:(g + 1) * w3grp],
                                          dr[:, g * w3grp:(g + 1) * w3grp])
            for tb in range(1, NTB):
                nc.sync.dma_start(xh_sb[:, tb], xh[:, tb])
                nc.sync.dma_start(xl_sb[:, tb], xl[:, tb])

            for tb in range(NTB):
                g_hi = gpool.tile([P, FC, TBS], e4, name=f"ghi_{tb}", tag="ghi")
                g_lo = gpool.tile([P, FC, TBS], e4, name=f"glo_{tb}", tag="glo")

                # phase A: GEMM1 (3 fp8 terms, one PSUM group) + GELU + requant
                for fc in range(FC):
                    h_ps = hpsum.tile([P, TBS], f32, name=f"h_{tb}_{fc}", tag="h")
                    terms = (
                        (w0h_sb, xh_sb),
                        (w0h_sb, xl_sb),
                        (w0l_sb, xh_sb),
                    )
                    n = len(terms) * DPAIR
                    k = 0
                    for wt, xt in terms:
                        for j in range(DPAIR):
                            nc.tensor.matmul(
                                h_ps[:],
                                wt[:, fc, j],
                                xt[:, tb, j],
                                start=(k == 0),
                                stop=(k == n - 1),
                                perf_mode=PM.DoubleRow,
                            )
                            k += 1
                    gf = gfpool.tile([P, TBS], bf16, name=f"gf_{tb}_{fc}", tag="gf")
                    nc.scalar.activation(gf[:], h_ps[:], AFT.Gelu, scale=1.0 / SW)
                    nc.scalar.activation(g_hi[:, fc], h_ps[:], AFT.Gelu,
                                         scale=1.0 / SW)
                    nc.vector.scalar_tensor_tensor(g_lo[:, fc], g_hi[:, fc], -1.0,
                                                   gf[:], op0=ALU.mult, op1=ALU.add)

                # phase B: GEMM2 (3 fp8 terms, one PSUM group per out tile)
                for ts in range(NTS):
                    for dc in range(ND):
                        o_ps = opsum.tile([P, DW], f32, name=f"o_{tb}_{ts}_{dc}",
                                          tag="o")
                        d0 = dc * DW
                        nmm = FPAIR * 3
                        k = 0
                        for jj in range(FPAIR):
                            gh = g_hi[:, 2 * jj:2 * jj + 2, ts * P:(ts + 1) * P]
                            gl = g_lo[:, 2 * jj:2 * jj + 2, ts * P:(ts + 1) * P]
                            for lhs, rhs in ((gh, w3h_sb), (gl, w3h_sb),
                                             (gh, w3l_sb)):
                                nc.tensor.matmul(
                                    o_ps[:],
                                    lhs,
                                    rhs[:, jj, :, d0:d0 + DW],
                                    start=(k == 0),
                                    stop=(k == nmm - 1),
                                    perf_mode=PM.DoubleRow,
                                )
                                k += 1
                        o_sb = opool.tile([P, DW], f32, name=f"os_{tb}_{ts}_{dc}",
                                          tag="os")
                        nc.vector.tensor_scalar_mul(o_sb[:], o_ps[:], 1.0 / SW)
                        nc.sync.dma_start(
                            out[tb * TBS + ts * P: tb * TBS + (ts + 1) * P,
                                d0:d0 + DW],
                            o_sb[:],
                        )

    nc.compile()
    return nc


def _get_nc():
    if "nc" not in _cache:
        _cache["nc"] = _build_nc_fp8()
    return _cache["nc"]


def _make_cached_fn(nc):
    """Build a reusable jitted 8-core executable around bass2jax's bass_exec
    primitive (the same lowering run_bass_kernel_spmd uses under axon), so
    repeat kernel() calls skip retrace/relower."""
    import jax
    import numpy as np
    from jax.sharding import Mesh, PartitionSpec
    try:
        from jax.experimental.shard_map import shard_map
    except ImportError:
        from jax.shard_map import shard_map
    import concourse.mybir as mybir
    from concourse.bass2jax import (_bass_exec_p, install_neuronx_cc_hook,
                                    partition_id_tensor)

    install_neuronx_cc_hook()
    partition_name = nc.partition_id_tensor.name if nc.partition_id_tensor else None
    in_names, out_names, out_avals, zero_shapes = [], [], [], []
    for alloc in nc.m.functions[0].allocations:
        if not isinstance(alloc, mybir.MemoryLocationSet):
            continue
        name = alloc.memorylocations[0].name
        if alloc.kind == "ExternalInput":
            if name != partition_name:
                in_names.append(name)
        elif alloc.kind == "ExternalOutput":
            out_names.append(name)
            shape = tuple(alloc.tensor_shape)
            dtype = mybir.dt.np(alloc.dtype)
            out_avals.append(jax.core.ShapedArray(shape, dtype))
            zero_shapes.append((shape, dtype))
    n_params = len(in_names)
    all_in_names = list(in_names) + list(out_names)
    if partition_name is not None:
        all_in_names.append(partition_name)

    def _body(*args):
        ins = list(args[:n_params])
        outs = list(args[n_params:])
        extra = [partition_id_tensor()] if partition_name is not None else []
        return tuple(_bass_exec_p.bind(
            *ins, *outs, *extra,
            out_avals=tuple(out_avals),
            in_names=tuple(all_in_names),
            out_names=tuple(out_names),
            lowering_input_output_aliases=(),
            sim_require_finite=True,
            sim_require_nnan=True,
            nc=nc,
        ))

    devices = jax.devices()[:N_CORES]
    mesh = Mesh(np.asarray(devices), ("core",))
    fn = jax.jit(
        shard_map(_body, mesh=mesh,
                  in_specs=(PartitionSpec("core"),) * (n_params + len(out_names)),
                  out_specs=(PartitionSpec("core"),) * len(out_names),
                  check_rep=False),
        keep_unused=True)

    def run(in_maps):
        concat_in = [np.concatenate([np.asarray(m[n]) for m in in_maps], axis=0)
                     for n in in_names]
        concat_zeros = [np.zeros((N_CORES * s[0], *s[1:]), dt)
                        for s, dt in zero_shapes]
        outs = fn(*concat_in, *concat_zeros)
        return [
            {name: np.asarray(outs[i]).reshape(N_CORES, *out_avals[i].shape)[c]
             for i, name in enumerate(out_names)}
            for c in range(N_CORES)
        ]

    return run


def kernel(**inputs):
    import os
    import sys
    if "/opt/trn_rl_repo" not in sys.path:
        sys.path.insert(0, "/opt/trn_rl_repo")
    from concourse import bass_utils

    output_tensor = np.asarray(inputs["output_tensor"], dtype=np.float32)  # [1, 8]
    x = np.asarray(inputs["inputs"], dtype=np.float32)   # [1, 8, 2048, 1024]
    w0 = np.asarray(inputs["w0"], dtype=np.float32)      # [8, 4096, 1024]
    w3 = np.asarray(inputs["w3"], dtype=np.float32)      # [8, 1024, 4096]

    e4 = ml_dtypes.float8_e4m3

    def prep_expert(e):
        # hi/lo e4m3 decomposition; weights pre-scaled by SW so both parts
        # stay clear of the e4m3 subnormal floor (see module docstring).
        xe = x[0, e]
        xh8 = xe.astype(e4)
        xl8 = (xe - xh8.astype(np.float32)).astype(e4)
        w0s = w0[e] * np.float32(SW)
        w0h8 = w0s.astype(e4)
        w0l8 = (w0s - w0h8.astype(np.float32)).astype(e4)
        w3s = w3[e] * np.float32(SW)
        w3h8 = w3s.astype(e4)
        w3l8 = (w3s - w3h8.astype(np.float32)).astype(e4)

        def lay_x(a):      # [T, D] -> [P, NTB, DPAIR, 2, TBS]
            return np.ascontiguousarray(
                a.reshape(NTB, TBS, 2 * DPAIR, P).transpose(3, 0, 2, 1)
                .reshape(P, NTB, DPAIR, 2, TBS))

        def lay_w0(a):     # [F, D] -> [P, FC, DPAIR, 2, P]
            return np.ascontiguousarray(
                a.reshape(FC, P, 2 * DPAIR, P).transpose(3, 0, 2, 1)
                .reshape(P, FC, DPAIR, 2, P))

        def lay_w3(a):     # [D, F] -> [P, FPAIR, 2, D]
            return np.ascontiguousarray(
                a.T.reshape(2 * FPAIR, P, D).transpose(1, 0, 2)
                .reshape(P, FPAIR, 2, D))

        return {
            "xh": lay_x(xh8), "xl": lay_x(xl8),
            "w0h": lay_w0(w0h8), "w0l": lay_w0(w0l8),
            "w3h": lay_w3(w3h8), "w3l": lay_w3(w3l8),
        }

    from concurrent.futures import ThreadPoolExecutor
    with ThreadPoolExecutor(max_workers=N_CORES) as pool:
        in_maps = list(pool.map(prep_expert, range(N_CORES)))

    nc = _get_nc()
    results = None
    if "fast_fn" in _cache:
        try:
            results = _cache["fast_fn"](in_maps)
        except Exception:
            results = None
    if results is None:
        try:
            results = bass_utils.run_bass_kernel_spmd(
                nc, in_maps, core_ids=list(range(N_CORES))).results
        except ModuleNotFoundError:
            # trace path requested via env but axon NTFF hook missing
            os.environ["BASS_NEVER_TRACE"] = "1"
            results = bass_utils.run_bass_kernel_spmd(
                nc, in_maps, core_ids=list(range(N_CORES))).results
        try:
            fast = _make_cached_fn(nc)
            fast(in_maps)  # warm: jit trace + XLA/NEFF compile happens here
            _cache["fast_fn"] = fast
        except Exception:
            pass
    out_full = np.stack([results[e]["out"] for e in range(N_CORES)])[None]

    # unpopular experts with zero gating activity produce zeros
    unpop = output_tensor[:, NUM_LOCAL:].sum(axis=0) != 0
    mask = np.concatenate([np.ones(NUM_LOCAL, dtype=bool), unpop])
    out_full = out_full * mask[None, :, None, None].astype(np.float32)
    return out_full.astype(np.float32)


# revision 8
# speedup vs baseline: 1.2576x; 1.0568x over previous
"""Trainium2 Bass kernel for nn_Experts (grouped MoE expert MLP).

Computes, for each of 8 experts e:
    h   = x_e @ w0_e.T          # [2048,1024] @ [1024,4096] -> [2048,4096]
    g   = gelu_exact(h)
    out = g @ w3_e.T            # [2048,4096] @ [4096,1024] -> [2048,1024]
then masks unpopular experts with zero gating activity (output_tensor).

Sharding: expert-parallel, 1 expert per NeuronCore across 8 cores (SPMD —
one compiled NEFF, per-core input data).

Numerics/perf strategy: fp8 (e4m3) matmuls in DoubleRow perf mode (K=256 per
matmul, 0.5 cycles/row) with residual-corrected operands. Every GEMM is
evaluated as three fp8 term-GEMMs accumulated in ONE PSUM group:

    x @ W ~= X_hi @ W_hi + X_lo @ W_hi + X_hi @ W_lo

where X_hi = e4m3(x), X_lo = e4m3(x - X_hi) (the residual is representable
unscaled because x ~ N(0,1)), and W is pre-scaled by SW=64 on the host so
BOTH its hi part and its residual stay clear of e4m3's subnormal floor. All
three terms then share the same global scale (SW), so they can accumulate
into a single PSUM bank with no combine pass; the SW descale folds into the
GELU activation's input scale (GEMM1) or the output copy's scale (GEMM2).
Measured end-to-end rel err of this scheme: ~2.6e-3 (limit 2e-2).

g is re-quantized the same way: gelu writes G_hi = e4m3(g) and a bf16 copy
g_f; the DVE computes G_lo = e4m3(g_f - G_hi) in one scalar_tensor_tensor op.
"""

import numpy as np
import ml_dtypes

T = 2048      # tokens (capacity) per expert
D = 1024      # hidden
F = 4096      # ffn
P = 128       # partitions
TBS = 256     # token block (GEMM1 moving free dim = 2*256 DR-packed)
NTB = T // TBS        # 8
DPAIR = D // (2 * P)  # 4  k-pairs in GEMM1 contraction
FPAIR = F // (2 * P)  # 16 k-pairs in GEMM2 contraction
FC = F // P           # 32 f-chunks (GEMM1 output tiles per token block)
DW = 256              # GEMM2 output free-dim chunk
ND = D // DW          # 4
NTS = TBS // P        # 2 t-subblocks per token block
SW = 64.0             # global weight pre-scale (power of 2)
NUM_LOCAL = 4
N_CORES = 8

_cache = {}


def _build_nc_fp8(
    g_bufs=2,        # generations of the per-block G_hi/G_lo tiles
    gf_bufs=3,       # gelu scratch ring ([128, FCP, 256] bf16 packs)
    h_bufs=2,        # GEMM1 PSUM packs in flight ([128, FCP, 256] = 2 banks)
    o_ps_bufs=3,     # GEMM2 PSUM tiles in flight
    o_sb_bufs=5,     # output staging ring
    warmup_mms=28,   # scratch matmuls riding out the PE cold-clock window
                     # while the initial DMAs land
    wgrp=8,          # fc per W0 DMA chunk
    w3grp=4,         # jj per W3 DMA chunk
    fcp=4,           # fc tiles per PSUM pack / GELU activation (amortizes the
                     # per-instruction activation init cost; keeps the Act
                     # engine under the PE group rate in phase A)
):
    import sys
    if "/opt/trn_rl_repo" not in sys.path:
        sys.path.insert(0, "/opt/trn_rl_repo")
    import concourse.tile as tile
    import concourse.mybir as mybir
    from concourse import bacc

    bf16 = mybir.dt.bfloat16
    f32 = mybir.dt.float32
    e4 = mybir.dt.float8e4
    AFT = mybir.ActivationFunctionType
    PM = mybir.MatmulPerfMode
    ALU = mybir.AluOpType

    nc = bacc.Bacc(
        "TRN2",
        target_bir_lowering=False,
        debug=False,
        enable_asserts=True,
        num_devices=N_CORES,
        dynamic_dma_scratch_size=2048,
    )

    # DRAM layouts (host-prepared, all contraction-major and DMA-contiguous):
    #   w0*[p, fc, j, i, m] = W0[f=fc*128+m, d=(2j+i)*128+p]
    #   x* [p, tb, j, i, t] = X [token=tb*256+t, d=(2j+i)*128+p]
    #   w3*[p, jj, i, dd]   = W3[dd, f=(2jj+i)*128+p]
    w0h = nc.dram_tensor("w0h", [P, FC, DPAIR, 2, P], e4, kind="ExternalInput").ap()
    w0l = nc.dram_tensor("w0l", [P, FC, DPAIR, 2, P], e4, kind="ExternalInput").ap()
    xh = nc.dram_tensor("xh", [P, NTB, DPAIR, 2, TBS], e4, kind="ExternalInput").ap()
    xl = nc.dram_tensor("xl", [P, NTB, DPAIR, 2, TBS], e4, kind="ExternalInput").ap()
    w3h = nc.dram_tensor("w3h", [P, FPAIR, 2, D], e4, kind="ExternalInput").ap()
    w3l = nc.dram_tensor("w3l", [P, FPAIR, 2, D], e4, kind="ExternalInput").ap()
    out = nc.dram_tensor("out", [T, D], f32, kind="ExternalOutput").ap()

    with tile.TileContext(nc) as tc:
        with (
            tc.tile_pool(name="weights", bufs=1) as wpool,
            tc.tile_pool(name="gtiles", bufs=g_bufs) as gpool,
            tc.tile_pool(name="gf", bufs=gf_bufs) as gfpool,
            tc.tile_pool(name="ostage", bufs=o_sb_bufs) as opool,
            tc.tile_pool(name="hps", bufs=h_bufs, space="PSUM") as hpsum,
            tc.tile_pool(name="ops", bufs=o_ps_bufs, space="PSUM") as opsum,
        ):
            w0h_sb = wpool.tile([P, FC, DPAIR, 2, P], e4, name="w0h_sb", tag="w0h")
            w0l_sb = wpool.tile([P, FC, DPAIR, 2, P], e4, name="w0l_sb", tag="w0l")
            xh_sb = wpool.tile([P, NTB, DPAIR, 2, TBS], e4, name="xh_sb", tag="xh")
            xl_sb = wpool.tile([P, NTB, DPAIR, 2, TBS], e4, name="xl_sb", tag="xl")
            w3h_sb = wpool.tile([P, FPAIR, 2, D], e4, name="w3h_sb", tag="w3h")
            w3l_sb = wpool.tile([P, FPAIR, 2, D], e4, name="w3l_sb", tag="w3l")

            if warmup_mms:
                with (
                    tc.tile_pool(name="warm", bufs=1) as warmpool,
                    tc.tile_pool(name="warmps", bufs=1, space="PSUM") as warmpsum,
                ):
                    wsrc = warmpool.tile([P, 512], bf16, name="wsrc", tag="wsrc")
                    wps = warmpsum.tile([P, 512], f32, name="wps", tag="wps")
                    nc.gpsimd.memset(wsrc[:], 0.0)
                    for i in range(warmup_mms):
                        nc.tensor.matmul(wps[:], wsrc[:, :P], wsrc[:],
                                         start=(i == 0), stop=(i == warmup_mms - 1))

            # DMA issue order = consumption order: x(tb0) first, then W0/W3
            # chunks interleaved so phase A's fc-groups and phase B's jj-groups
            # arrive just ahead of the PE, then the remaining token blocks.
            nc.sync.dma_start(xh_sb[:, 0], xh[:, 0])
            nc.sync.dma_start(xl_sb[:, 0], xl[:, 0])
            w0_chunks = [(w0h_sb, w0h), (w0l_sb, w0l)]
            w3_chunks = [(w3h_sb, w3h), (w3l_sb, w3l)]
            NWG, NW3G = FC // wgrp, FPAIR // w3grp
            sched = []
            for g in range(NWG):
                sched.append(("w0", g))
                if g >= 1 and (g - 1) < NW3G:
                    sched.append(("w3", g - 1))
            for g in range(NWG - 1, NW3G):
                sched.append(("w3", g))
            for kind, g in sched:
                if kind == "w0":
                    for sb_t, dr in w0_chunks:
                        nc.sync.dma_start(sb_t[:, g * wgrp:(g + 1) * wgrp],
                                          dr[:, g * wgrp:(g + 1) * wgrp])
                else:
                    for sb_t, dr in w3_chunks:
                        nc.sync.dma_start(sb_t[:, g * w3grp:(g + 1) * w3grp],
                                          dr[:, g * w3grp:(g + 1) * w3grp])
                    if g == 1:
                        # x(tb1) must land before phase A of tb1 (~41us in)
                        nc.sync.dma_start(xh_sb[:, 1], xh[:, 1])
                        nc.sync.dma_start(xl_sb[:, 1], xl[:, 1])
            for tb in range(2, NTB):
                nc.sync.dma_start(xh_sb[:, tb], xh[:, tb])
                nc.sync.dma_start(xl_sb[:, tb], xl[:, tb])

            for tb in range(NTB):
                g_hi = gpool.tile([P, FC, TBS], e4, name=f"ghi_{tb}", tag="ghi")
                g_lo = gpool.tile([P, FC, TBS], e4, name=f"glo_{tb}", tag="glo")

                # phase A: GEMM1 (3 fp8 terms, one PSUM group per fc) + GELU +
                # requant. fcp fc-tiles share one PSUM pack so the activations
                # and the requant run as wide ops (init cost amortized).
                terms = (
                    (w0h_sb, xh_sb),
                    (w0h_sb, xl_sb),
                    (w0l_sb, xh_sb),
                )
                nmm1 = len(terms) * DPAIR
                for fp in range(FC // fcp):
                    h_ps = hpsum.tile([P, fcp, TBS], f32, name=f"h_{tb}_{fp}",
                                      tag="h")
                    for s in range(fcp):
                        fc = fp * fcp + s
                        k = 0
                        for wt, xt in terms:
                            for j in range(DPAIR):
                                nc.tensor.matmul(
                                    h_ps[:, s],
                                    wt[:, fc, j],
                                    xt[:, tb, j],
                                    start=(k == 0),
                                    stop=(k == nmm1 - 1),
                                    perf_mode=PM.DoubleRow,
                                )
                                k += 1
                    gf = gfpool.tile([P, fcp, TBS], bf16, name=f"gf_{tb}_{fp}",
                                     tag="gf")
                    gslc = slice(fp * fcp, (fp + 1) * fcp)
                    nc.scalar.activation(gf[:], h_ps[:], AFT.Gelu, scale=1.0 / SW)
                    nc.scalar.activation(g_hi[:, gslc], h_ps[:], AFT.Gelu,
                                         scale=1.0 / SW)
                    nc.vector.scalar_tensor_tensor(g_lo[:, gslc], g_hi[:, gslc],
                                                   -1.0, gf[:],
                                                   op0=ALU.mult, op1=ALU.add)

                # phase B: GEMM2 (3 fp8 terms, one PSUM group per out tile)
                for ts in range(NTS):
                    for dc in range(ND):
                        o_ps = opsum.tile([P, DW], f32, name=f"o_{tb}_{ts}_{dc}",
                                          tag="o")
                        d0 = dc * DW
                        nmm = FPAIR * 3
                        k = 0
                        for jj in range(FPAIR):
                            gh = g_hi[:, 2 * jj:2 * jj + 2, ts * P:(ts + 1) * P]
                            gl = g_lo[:, 2 * jj:2 * jj + 2, ts * P:(ts + 1) * P]
                            for lhs, rhs in ((gh, w3h_sb), (gl, w3h_sb),
                                             (gh, w3l_sb)):
                                nc.tensor.matmul(
                                    o_ps[:],
                                    lhs,
                                    rhs[:, jj, :, d0:d0 + DW],
                                    start=(k == 0),
                                    stop=(k == nmm - 1),
                                    perf_mode=PM.DoubleRow,
                                )
                                k += 1
                        o_sb = opool.tile([P, DW], f32, name=f"os_{tb}_{ts}_{dc}",
                                          tag="os")
                        nc.vector.tensor_scalar_mul(o_sb[:], o_ps[:], 1.0 / SW)
                        nc.sync.dma_start(
                            out[tb * TBS + ts * P: tb * TBS + (ts + 1) * P,
                                d0:d0 + DW],
                            o_sb[:],
                        )

    nc.compile()
    return nc


def _get_nc():
    if "nc" not in _cache:
        _cache["nc"] = _build_nc_fp8()
    return _cache["nc"]


def _make_cached_fn(nc):
    """Build a reusable jitted 8-core executable around bass2jax's bass_exec
    primitive (the same lowering run_bass_kernel_spmd uses under axon), so
    repeat kernel() calls skip retrace/relower."""
    import jax
    import numpy as np
    from jax.sharding import Mesh, PartitionSpec
    try:
        from jax.experimental.shard_map import shard_map
    except ImportError:
        from jax.shard_map import shard_map
    import concourse.mybir as mybir
    from concourse.bass2jax import (_bass_exec_p, install_neuronx_cc_hook,
                                    partition_id_tensor)

    install_neuronx_cc_hook()
    partition_name = nc.partition_id_tensor.name if nc.partition_id_tensor else None
    in_names, out_names, out_avals, zero_shapes = [], [], [], []
    for alloc in nc.m.functions[0].allocations:
        if not isinstance(alloc, mybir.MemoryLocationSet):
            continue
        name = alloc.memorylocations[0].name
        if alloc.kind == "ExternalInput":
            if name != partition_name:
                in_names.append(name)
        elif alloc.kind == "ExternalOutput":
            out_names.append(name)
            shape = tuple(alloc.tensor_shape)
            dtype = mybir.dt.np(alloc.dtype)
            out_avals.append(jax.core.ShapedArray(shape, dtype))
            zero_shapes.append((shape, dtype))
    n_params = len(in_names)
    all_in_names = list(in_names) + list(out_names)
    if partition_name is not None:
        all_in_names.append(partition_name)

    def _body(*args):
        ins = list(args[:n_params])
        outs = list(args[n_params:])
        extra = [partition_id_tensor()] if partition_name is not None else []
        return tuple(_bass_exec_p.bind(
            *ins, *outs, *extra,
            out_avals=tuple(out_avals),
            in_names=tuple(all_in_names),
            out_names=tuple(out_names),
            lowering_input_output_aliases=(),
            sim_require_finite=True,
            sim_require_nnan=True,
            nc=nc,
        ))

    devices = jax.devices()[:N_CORES]
    mesh = Mesh(np.asarray(devices), ("core",))
    fn = jax.jit(
        shard_map(_body, mesh=mesh,
                  in_specs=(PartitionSpec("core"),) * (n_params + len(out_names)),
                  out_specs=(PartitionSpec("core"),) * len(out_names),
                  check_rep=False),
        keep_unused=True)

    def run(in_maps):
        concat_in = [np.concatenate([np.asarray(m[n]) for m in in_maps], axis=0)
                     for n in in_names]
        concat_zeros = [np.zeros((N_CORES * s[0], *s[1:]), dt)
                        for s, dt in zero_shapes]
        outs = fn(*concat_in, *concat_zeros)
        return [
            {name: np.asarray(outs[i]).reshape(N_CORES, *out_avals[i].shape)[c]
             for i, name in enumerate(out_names)}
            for c in range(N_CORES)
        ]

    return run


def kernel(**inputs):
    import os
    import sys
    if "/opt/trn_rl_repo" not in sys.path:
        sys.path.insert(0, "/opt/trn_rl_repo")
    from concourse import bass_utils

    output_tensor = np.asarray(inputs["output_tensor"], dtype=np.float32)  # [1, 8]
    x = np.asarray(inputs["inputs"], dtype=np.float32)   # [1, 8, 2048, 1024]
    w0 = np.asarray(inputs["w0"], dtype=np.float32)      # [8, 4096, 1024]
    w3 = np.asarray(inputs["w3"], dtype=np.float32)      # [8, 1024, 4096]

    e4 = ml_dtypes.float8_e4m3

    def prep_expert(e):
        # hi/lo e4m3 decomposition; weights pre-scaled by SW so both parts
        # stay clear of the e4m3 subnormal floor (see module docstring).
        xe = x[0, e]
        xh8 = xe.astype(e4)
        xl8 = (xe - xh8.astype(np.float32)).astype(e4)
        w0s = w0[e] * np.float32(SW)
        w0h8 = w0s.astype(e4)
        w0l8 = (w0s - w0h8.astype(np.float32)).astype(e4)
        w3s = w3[e] * np.float32(SW)
        w3h8 = w3s.astype(e4)
        w3l8 = (w3s - w3h8.astype(np.float32)).astype(e4)

        def lay_x(a):      # [T, D] -> [P, NTB, DPAIR, 2, TBS]
            return np.ascontiguousarray(
                a.reshape(NTB, TBS, 2 * DPAIR, P).transpose(3, 0, 2, 1)
                .reshape(P, NTB, DPAIR, 2, TBS))

        def lay_w0(a):     # [F, D] -> [P, FC, DPAIR, 2, P]
            return np.ascontiguousarray(
                a.reshape(FC, P, 2 * DPAIR, P).transpose(3, 0, 2, 1)
                .reshape(P, FC, DPAIR, 2, P))

        def lay_w3(a):     # [D, F] -> [P, FPAIR, 2, D]
            return np.ascontiguousarray(
                a.T.reshape(2 * FPAIR, P, D).transpose(1, 0, 2)
                .reshape(P, FPAIR, 2, D))

        return {
            "xh": lay_x(xh8), "xl": lay_x(xl8),
            "w0h": lay_w0(w0h8), "w0l": lay_w0(w0l8),
            "w3h": lay_w3(w3h8), "w3l": lay_w3(w3l8),
        }

    from concurrent.futures import ThreadPoolExecutor
    with ThreadPoolExecutor(max_workers=N_CORES) as pool:
        in_maps = list(pool.map(prep_expert, range(N_CORES)))

    nc = _get_nc()
    results = None
    if "fast_fn" in _cache:
        try:
            results = _cache["fast_fn"](in_maps)
        except Exception:
            results = None
    if results is None:
        try:
            results = bass_utils.run_bass_kernel_spmd(
                nc, in_maps, core_ids=list(range(N_CORES))).results
        except ModuleNotFoundError:
            # trace path requested via env but axon NTFF hook missing
            os.environ["BASS_NEVER_TRACE"] = "1"
            results = bass_utils.run_bass_kernel_spmd(
                nc, in_maps, core_ids=list(range(N_CORES))).results
        try:
            fast = _make_cached_fn(nc)
            fast(in_maps)  # warm: jit trace + XLA/NEFF compile happens here
            _cache["fast_fn"] = fast
        except Exception:
            pass
    out_full = np.stack([results[e]["out"] for e in range(N_CORES)])[None]

    # unpopular experts with zero gating activity produce zeros
    unpop = output_tensor[:, NUM_LOCAL:].sum(axis=0) != 0
    mask = np.concatenate([np.ones(NUM_LOCAL, dtype=bool), unpop])
    out_full = out_full * mask[None, :, None, None].astype(np.float32)
    return out_full.astype(np.float32)


# revision 10
# speedup vs baseline: 1.3074x; 1.0396x over previous
"""Trainium2 Bass kernel for nn_Experts (grouped MoE expert MLP).

Computes, for each of 8 experts e:
    h   = x_e @ w0_e.T          # [2048,1024] @ [1024,4096] -> [2048,4096]
    g   = gelu_exact(h)
    out = g @ w3_e.T            # [2048,4096] @ [4096,1024] -> [2048,1024]
then masks unpopular experts with zero gating activity (output_tensor).

Sharding: expert-parallel, 1 expert per NeuronCore across 8 cores (SPMD —
one compiled NEFF, per-core input data).

Numerics/perf strategy: fp8 (e4m3) matmuls in DoubleRow perf mode (K=256 per
matmul, 0.5 cycles/row) with residual-corrected operands. Every GEMM is
evaluated as three fp8 term-GEMMs accumulated in ONE PSUM group:

    x @ W ~= X_hi @ W_hi + X_lo @ W_hi + X_hi @ W_lo

where X_hi = e4m3(x), X_lo = e4m3(x - X_hi) (the residual is representable
unscaled because x ~ N(0,1)), and W is pre-scaled by SW=64 on the host so
BOTH its hi part and its residual stay clear of e4m3's subnormal floor. All
three terms then share the same global scale (SW), so they can accumulate
into a single PSUM bank with no combine pass; the SW descale folds into the
GELU activation's input scale (GEMM1) or the output copy's scale (GEMM2).
Measured end-to-end rel err of this scheme: ~2.6e-3 (limit 2e-2).

g is re-quantized the same way: gelu writes G_hi = e4m3(g) and a bf16 copy
g_f; the DVE computes G_lo = e4m3(g_f - G_hi) in one scalar_tensor_tensor op.
"""

import numpy as np
import ml_dtypes

T = 2048      # tokens (capacity) per expert
D = 1024      # hidden
F = 4096      # ffn
P = 128       # partitions
TBS = 256     # token block (GEMM1 moving free dim = 2*256 DR-packed)
NTB = T // TBS        # 8
DPAIR = D // (2 * P)  # 4  k-pairs in GEMM1 contraction
FPAIR = F // (2 * P)  # 16 k-pairs in GEMM2 contraction
FC = F // P           # 32 f-chunks (GEMM1 output tiles per token block)
DW = 256              # GEMM2 output free-dim chunk
ND = D // DW          # 4
NTS = TBS // P        # 2 t-subblocks per token block
SW = 64.0             # global weight pre-scale (power of 2)
NUM_LOCAL = 4
N_CORES = 8

_cache = {}


def _build_nc_fp8(
    g_bufs=3,        # generations of the per-block G_hi/G_lo tiles (phase A
                     # runs one token block ahead of phase B)
    x_bufs=4,        # generations of the streamed per-block x tiles
    gf_bufs=3,       # gelu scratch ring ([128, FCP, 256] bf16 packs)
    h_bufs=2,        # GEMM1 PSUM packs in flight ([128, FCP, 256] = 2 banks)
    o_ps_bufs=4,     # GEMM2 PSUM tiles in flight
    o_sb_bufs=5,     # output staging ring
    warmup_mms=14,   # scratch matmuls riding out the PE cold-clock window
                     # while the initial DMAs land
    wgrp=4,          # fc per W0 DMA chunk
    w3grp=4,         # jj per W3 DMA chunk
    fcp=4,           # fc tiles per PSUM pack / GELU activation (amortizes the
                     # per-instruction activation init cost; keeps the Act
                     # engine under the PE group rate in phase A)
):
    import sys
    if "/opt/trn_rl_repo" not in sys.path:
        sys.path.insert(0, "/opt/trn_rl_repo")
    import concourse.tile as tile
    import concourse.mybir as mybir
    from concourse import bacc

    bf16 = mybir.dt.bfloat16
    f32 = mybir.dt.float32
    e4 = mybir.dt.float8e4
    AFT = mybir.ActivationFunctionType
    PM = mybir.MatmulPerfMode
    ALU = mybir.AluOpType

    nc = bacc.Bacc(
        "TRN2",
        target_bir_lowering=False,
        debug=False,
        enable_asserts=True,
        num_devices=N_CORES,
        dynamic_dma_scratch_size=2048,
    )

    # DRAM layouts (host-prepared, all contraction-major and DMA-contiguous):
    #   w0*[p, fc, j, i, m] = W0[f=fc*128+m, d=(2j+i)*128+p]
    #   x* [p, tb, j, i, t] = X [token=tb*256+t, d=(2j+i)*128+p]
    #   w3*[p, jj, i, dd]   = W3[dd, f=(2jj+i)*128+p]
    w0h = nc.dram_tensor("w0h", [P, FC, DPAIR, 2, P], e4, kind="ExternalInput").ap()
    w0l = nc.dram_tensor("w0l", [P, FC, DPAIR, 2, P], e4, kind="ExternalInput").ap()
    xh = nc.dram_tensor("xh", [P, NTB, DPAIR, 2, TBS], e4, kind="ExternalInput").ap()
    xl = nc.dram_tensor("xl", [P, NTB, DPAIR, 2, TBS], e4, kind="ExternalInput").ap()
    w3h = nc.dram_tensor("w3h", [P, FPAIR, 2, D], e4, kind="ExternalInput").ap()
    w3l = nc.dram_tensor("w3l", [P, FPAIR, 2, D], e4, kind="ExternalInput").ap()
    out = nc.dram_tensor("out", [T, D], f32, kind="ExternalOutput").ap()

    with tile.TileContext(nc) as tc:
        with (
            tc.tile_pool(name="weights", bufs=1) as wpool,
            tc.tile_pool(name="xin", bufs=x_bufs) as xpool,
            tc.tile_pool(name="gtiles", bufs=g_bufs) as gpool,
            tc.tile_pool(name="gf", bufs=gf_bufs) as gfpool,
            tc.tile_pool(name="ostage", bufs=o_sb_bufs) as opool,
        ):
            w0h_sb = wpool.tile([P, FC, DPAIR, 2, P], e4, name="w0h_sb", tag="w0h")
            w0l_sb = wpool.tile([P, FC, DPAIR, 2, P], e4, name="w0l_sb", tag="w0l")
            w3h_sb = wpool.tile([P, FPAIR, 2, D], e4, name="w3h_sb", tag="w3h")
            w3l_sb = wpool.tile([P, FPAIR, 2, D], e4, name="w3l_sb", tag="w3l")

            if warmup_mms:
                # separate scope: its PSUM bank is released before the main
                # PSUM pools open
                with (
                    tc.tile_pool(name="warm", bufs=1) as warmpool,
                    tc.tile_pool(name="warmps", bufs=1, space="PSUM") as warmpsum,
                ):
                    wsrc = warmpool.tile([P, 512], bf16, name="wsrc", tag="wsrc")
                    wps = warmpsum.tile([P, 512], f32, name="wps", tag="wps")
                    nc.gpsimd.memset(wsrc[:], 0.0)
                    for i in range(warmup_mms):
                        nc.tensor.matmul(wps[:], wsrc[:, :P], wsrc[:],
                                         start=(i == 0), stop=(i == warmup_mms - 1))

            # streamed x tiles, one pair per token block
            x_tiles = {}
            def load_x(tb):
                xht = xpool.tile([P, DPAIR, 2, TBS], e4, name=f"xh_{tb}", tag="xh")
                xlt = xpool.tile([P, DPAIR, 2, TBS], e4, name=f"xl_{tb}", tag="xl")
                nc.sync.dma_start(xht[:], xh[:, tb])
                nc.sync.dma_start(xlt[:], xl[:, tb])
                x_tiles[tb] = (xht, xlt)

            # DMA issue order = consumption order of the A0 A1 B0 A2 B1 ...
            # software pipeline: x(tb0), all W0 chunks (phase A0), x(tb1),
            # all W3 chunks (phase B0, which runs after A1), rest of x.
            load_x(0)
            for g in range(FC // wgrp):
                for sb_t, dr in ((w0h_sb, w0h), (w0l_sb, w0l)):
                    nc.sync.dma_start(sb_t[:, g * wgrp:(g + 1) * wgrp],
                                      dr[:, g * wgrp:(g + 1) * wgrp])
            load_x(1)
            for g in range(FPAIR // w3grp):
                for sb_t, dr in ((w3h_sb, w3h), (w3l_sb, w3l)):
                    nc.sync.dma_start(sb_t[:, g * w3grp:(g + 1) * w3grp],
                                      dr[:, g * w3grp:(g + 1) * w3grp])
            for tb in range(2, NTB):
                load_x(tb)

            with (
                tc.tile_pool(name="hps", bufs=h_bufs, space="PSUM") as hpsum,
                tc.tile_pool(name="ops", bufs=o_ps_bufs, space="PSUM") as opsum,
            ):
                g_pair = {}

                def phase_a(tb):
                    g_hi = gpool.tile([P, FC, TBS], e4, name=f"ghi_{tb}", tag="ghi")
                    g_lo = gpool.tile([P, FC, TBS], e4, name=f"glo_{tb}", tag="glo")
                    g_pair[tb] = (g_hi, g_lo)
                    xht, xlt = x_tiles.pop(tb)
                    # GEMM1: 3 fp8 terms, one PSUM group per fc; fcp fc-tiles
                    # share one PSUM pack so GELU + requant run as wide ops.
                    terms = ((w0h_sb, xht), (w0h_sb, xlt), (w0l_sb, xht))
                    nmm1 = len(terms) * DPAIR
                    for fp in range(FC // fcp):
                        h_ps = hpsum.tile([P, fcp, TBS], f32, name=f"h_{tb}_{fp}",
                                          tag="h")
                        for s in range(fcp):
                            fc = fp * fcp + s
                            k = 0
                            for wt, xt in terms:
                                for j in range(DPAIR):
                                    nc.tensor.matmul(
                                        h_ps[:, s],
                                        wt[:, fc, j],
                                        xt[:, j],
                                        start=(k == 0),
                                        stop=(k == nmm1 - 1),
                                        perf_mode=PM.DoubleRow,
                                    )
                                    k += 1
                        gf = gfpool.tile([P, fcp, TBS], bf16, name=f"gf_{tb}_{fp}",
                                         tag="gf")
                        gslc = slice(fp * fcp, (fp + 1) * fcp)
                        nc.scalar.activation(gf[:], h_ps[:], AFT.Gelu,
                                             scale=1.0 / SW)
                        nc.scalar.activation(g_hi[:, gslc], h_ps[:], AFT.Gelu,
                                             scale=1.0 / SW)
                        nc.vector.scalar_tensor_tensor(g_lo[:, gslc],
                                                       g_hi[:, gslc], -1.0,
                                                       gf[:], op0=ALU.mult,
                                                       op1=ALU.add)

                def phase_b(tb):
                    g_hi, g_lo = g_pair.pop(tb)
                    for ts in range(NTS):
                        for dc in range(ND):
                            o_ps = opsum.tile([P, DW], f32,
                                              name=f"o_{tb}_{ts}_{dc}", tag="o")
                            d0 = dc * DW
                            nmm = FPAIR * 3
                            k = 0
                            for jj in range(FPAIR):
                                gh = g_hi[:, 2 * jj:2 * jj + 2,
                                          ts * P:(ts + 1) * P]
                                gl = g_lo[:, 2 * jj:2 * jj + 2,
                                          ts * P:(ts + 1) * P]
                                for lhs, rhs in ((gh, w3h_sb), (gl, w3h_sb),
                                                 (gh, w3l_sb)):
                                    nc.tensor.matmul(
                                        o_ps[:],
                                        lhs,
                                        rhs[:, jj, :, d0:d0 + DW],
                                        start=(k == 0),
                                        stop=(k == nmm - 1),
                                        perf_mode=PM.DoubleRow,
                                    )
                                    k += 1
                            o_sb = opool.tile([P, DW], f32,
                                              name=f"os_{tb}_{ts}_{dc}", tag="os")
                            nc.vector.tensor_scalar_mul(o_sb[:], o_ps[:], 1.0 / SW)
                            nc.sync.dma_start(
                                out[tb * TBS + ts * P: tb * TBS + (ts + 1) * P,
                                    d0:d0 + DW],
                                o_sb[:],
                            )

                phase_a(0)
                for tb in range(1, NTB):
                    phase_a(tb)
                    phase_b(tb - 1)
                phase_b(NTB - 1)

    nc.compile()
    return nc


def _get_nc():
    if "nc" not in _cache:
        _cache["nc"] = _build_nc_fp8()
    return _cache["nc"]


def _make_cached_fn(nc):
    """Build a reusable jitted 8-core executable around bass2jax's bass_exec
    primitive (the same lowering run_bass_kernel_spmd uses under axon), so
    repeat kernel() calls skip retrace/relower."""
    import jax
    import numpy as np
    from jax.sharding import Mesh, PartitionSpec
    try:
        from jax.experimental.shard_map import shard_map
    except ImportError:
        from jax.shard_map import shard_map
    import concourse.mybir as mybir
    from concourse.bass2jax import (_bass_exec_p, install_neuronx_cc_hook,
                                    partition_id_tensor)

    install_neuronx_cc_hook()
    partition_name = nc.partition_id_tensor.name if nc.partition_id_tensor else None
    in_names, out_names, out_avals, zero_shapes = [], [], [], []
    for alloc in nc.m.functions[0].allocations:
        if not isinstance(alloc, mybir.MemoryLocationSet):
            continue
        name = alloc.memorylocations[0].name
        if alloc.kind == "ExternalInput":
            if name != partition_name:
                in_names.append(name)
        elif alloc.kind == "ExternalOutput":
            out_names.append(name)
            shape = tuple(alloc.tensor_shape)
            dtype = mybir.dt.np(alloc.dtype)
            out_avals.append(jax.core.ShapedArray(shape, dtype))
            zero_shapes.append((shape, dtype))
    n_params = len(in_names)
    all_in_names = list(in_names) + list(out_names)
    if partition_name is not None:
        all_in_names.append(partition_name)

    def _body(*args):
        ins = list(args[:n_params])
        outs = list(args[n_params:])
        extra = [partition_id_tensor()] if partition_name is not None else []
        return tuple(_bass_exec_p.bind(
            *ins, *outs, *extra,
            out_avals=tuple(out_avals),
            in_names=tuple(all_in_names),
            out_names=tuple(out_names),
            lowering_input_output_aliases=(),
            sim_require_finite=True,
            sim_require_nnan=True,
            nc=nc,
        ))

    devices = jax.devices()[:N_CORES]
    mesh = Mesh(np.asarray(devices), ("core",))
    fn = jax.jit(
        shard_map(_body, mesh=mesh,
                  in_specs=(PartitionSpec("core"),) * (n_params + len(out_names)),
                  out_specs=(PartitionSpec("core"),) * len(out_names),
                  check_rep=False),
        keep_unused=True)

    def run(in_maps):
        concat_in = [np.concatenate([np.asarray(m[n]) for m in in_maps], axis=0)
                     for n in in_names]
        concat_zeros = [np.zeros((N_CORES * s[0], *s[1:]), dt)
                        for s, dt in zero_shapes]
        outs = fn(*concat_in, *concat_zeros)
        return [
            {name: np.asarray(outs[i]).reshape(N_CORES, *out_avals[i].shape)[c]
             for i, name in enumerate(out_names)}
            for c in range(N_CORES)
        ]

    return run


def kernel(**inputs):
    import os
    import sys
    if "/opt/trn_rl_repo" not in sys.path:
        sys.path.insert(0, "/opt/trn_rl_repo")
    from concourse import bass_utils

    output_tensor = np.asarray(inputs["output_tensor"], dtype=np.float32)  # [1, 8]
    x = np.asarray(inputs["inputs"], dtype=np.float32)   # [1, 8, 2048, 1024]
    w0 = np.asarray(inputs["w0"], dtype=np.float32)      # [8, 4096, 1024]
    w3 = np.asarray(inputs["w3"], dtype=np.float32)      # [8, 1024, 4096]

    e4 = ml_dtypes.float8_e4m3

    def prep_expert(e):
        # hi/lo e4m3 decomposition; weights pre-scaled by SW so both parts
        # stay clear of the e4m3 subnormal floor (see module docstring).
        xe = x[0, e]
        xh8 = xe.astype(e4)
        xl8 = (xe - xh8.astype(np.float32)).astype(e4)
        w0s = w0[e] * np.float32(SW)
        w0h8 = w0s.astype(e4)
        w0l8 = (w0s - w0h8.astype(np.float32)).astype(e4)
        w3s = w3[e] * np.float32(SW)
        w3h8 = w3s.astype(e4)
        w3l8 = (w3s - w3h8.astype(np.float32)).astype(e4)

        def lay_x(a):      # [T, D] -> [P, NTB, DPAIR, 2, TBS]
            return np.ascontiguousarray(
                a.reshape(NTB, TBS, 2 * DPAIR, P).transpose(3, 0, 2, 1)
                .reshape(P, NTB, DPAIR, 2, TBS))

        def lay_w0(a):     # [F, D] -> [P, FC, DPAIR, 2, P]
            return np.ascontiguousarray(
                a.reshape(FC, P, 2 * DPAIR, P).transpose(3, 0, 2, 1)
                .reshape(P, FC, DPAIR, 2, P))

        def lay_w3(a):     # [D, F] -> [P, FPAIR, 2, D]
            return np.ascontiguousarray(
                a.T.reshape(2 * FPAIR, P, D).transpose(1, 0, 2)
                .reshape(P, FPAIR, 2, D))

        return {
            "xh": lay_x(xh8), "xl": lay_x(xl8),
            "w0h": lay_w0(w0h8), "w0l": lay_w0(w0l8),
            "w3h": lay_w3(w3h8), "w3l": lay_w3(w3l8),
        }

    from concurrent.futures import ThreadPoolExecutor
    with ThreadPoolExecutor(max_workers=N_CORES) as pool:
        in_maps = list(pool.map(prep_expert, range(N_CORES)))

    nc = _get_nc()
    results = None
    if "fast_fn" in _cache:
        try:
            results = _cache["fast_fn"](in_maps)
        except Exception:
            results = None
    if results is None:
        try:
            results = bass_utils.run_bass_kernel_spmd(
                nc, in_maps, core_ids=list(range(N_CORES))).results
        except ModuleNotFoundError:
            # trace path requested via env but axon NTFF hook missing
            os.environ["BASS_NEVER_TRACE"] = "1"
            results = bass_utils.run_bass_kernel_spmd(
                nc, in_maps, core_ids=list(range(N_CORES))).results
        try:
            fast = _make_cached_fn(nc)
            fast(in_maps)  # warm: jit trace + XLA/NEFF compile happens here
            _cache["fast_fn"] = fast
        except Exception:
            pass
    out_full = np.stack([results[e]["out"] for e in range(N_CORES)])[None]

    # unpopular experts with zero gating activity produce zeros
    unpop = output_tensor[:, NUM_LOCAL:].sum(axis=0) != 0
    mask = np.concatenate([np.ones(NUM_LOCAL, dtype=bool), unpop])
    out_full = out_full * mask[None, :, None, None].astype(np.float32)
    return out_full.astype(np.float32)


# revision 14
# speedup vs baseline: 1.3231x; 1.0119x over previous
"""Trainium2 Bass kernel for nn_Experts (grouped MoE expert MLP).

Computes, for each of 8 experts e:
    h   = x_e @ w0_e.T          # [2048,1024] @ [1024,4096] -> [2048,4096]
    g   = gelu_exact(h)
    out = g @ w3_e.T            # [2048,4096] @ [4096,1024] -> [2048,1024]
then masks unpopular experts with zero gating activity (output_tensor).

Sharding: expert-parallel, 1 expert per NeuronCore across 8 cores (SPMD —
one compiled NEFF, per-core input data).

Numerics/perf strategy: fp8 (e4m3) matmuls in DoubleRow perf mode (K=256 per
matmul, 0.5 cycles/row) with residual-corrected operands. Every GEMM is
evaluated as three fp8 term-GEMMs accumulated in ONE PSUM group:

    x @ W ~= X_hi @ W_hi + X_lo @ W_hi + X_hi @ W_lo

where X_hi = e4m3(x), X_lo = e4m3(x - X_hi) (the residual is representable
unscaled because x ~ N(0,1)), and W is pre-scaled by SW=64 on the host so
BOTH its hi part and its residual stay clear of e4m3's subnormal floor. All
three terms then share the same global scale (SW), so they can accumulate
into a single PSUM bank with no combine pass; the SW descale folds into the
GELU activation's input scale (GEMM1) or the output copy's scale (GEMM2).
Measured end-to-end rel err of this scheme: ~2.6e-3 (limit 2e-2).

g is re-quantized the same way: gelu writes G_hi = e4m3(g) and a bf16 copy
g_f; the DVE computes G_lo = e4m3(g_f - G_hi) in one scalar_tensor_tensor op.
"""

import numpy as np
import ml_dtypes

T = 2048      # tokens (capacity) per expert
D = 1024      # hidden
F = 4096      # ffn
P = 128       # partitions
TBS = 256     # token block (GEMM1 moving free dim = 2*256 DR-packed)
NTB = T // TBS        # 8
DPAIR = D // (2 * P)  # 4  k-pairs in GEMM1 contraction
FPAIR = F // (2 * P)  # 16 k-pairs in GEMM2 contraction
FC = F // P           # 32 f-chunks (GEMM1 output tiles per token block)
DW = 256              # GEMM2 output free-dim chunk
ND = D // DW          # 4
NTS = TBS // P        # 2 t-subblocks per token block
SW = 64.0             # global weight pre-scale (power of 2)
NUM_LOCAL = 4
N_CORES = 8

_cache = {}


def _build_nc_fp8(
    g_bufs=3,        # generations of the per-block G_hi/G_lo tiles (phase A
                     # runs one token block ahead of phase B)
    x_bufs=4,        # generations of the streamed per-block x tiles
    gf_bufs=3,       # gelu scratch ring ([128, FCP, 256] bf16 packs)
    h_bufs=3,        # GEMM1 PSUM packs in flight ([128, FCP, 256] = 2 banks)
    o_ps_bufs=2,     # GEMM2 PSUM tiles in flight
    o_sb_bufs=5,     # output staging ring
    warmup_mms=17,   # scratch matmuls riding out the PE cold-clock window
                     # while the initial DMAs land
    wgrp=4,          # fc per W0 DMA chunk
    w3grp=4,         # jj per W3 DMA chunk
    fcp=4,           # fc tiles per PSUM pack / GELU activation (amortizes the
                     # per-instruction activation init cost; keeps the Act
                     # engine under the PE group rate in phase A)
):
    import sys
    if "/opt/trn_rl_repo" not in sys.path:
        sys.path.insert(0, "/opt/trn_rl_repo")
    import concourse.tile as tile
    import concourse.mybir as mybir
    from concourse import bacc

    bf16 = mybir.dt.bfloat16
    f32 = mybir.dt.float32
    e4 = mybir.dt.float8e4
    AFT = mybir.ActivationFunctionType
    PM = mybir.MatmulPerfMode
    ALU = mybir.AluOpType

    nc = bacc.Bacc(
        "TRN2",
        target_bir_lowering=False,
        debug=False,
        enable_asserts=True,
        num_devices=N_CORES,
        dynamic_dma_scratch_size=2048,
    )

    # DRAM layouts (host-prepared, all contraction-major and DMA-contiguous):
    #   w0*[p, fc, j, i, m] = W0[f=fc*128+m, d=(2j+i)*128+p]
    #   x* [p, tb, j, i, t] = X [token=tb*256+t, d=(2j+i)*128+p]
    #   w3*[p, jj, i, dd]   = W3[dd, f=(2jj+i)*128+p]
    w0h = nc.dram_tensor("w0h", [P, FC, DPAIR, 2, P], e4, kind="ExternalInput").ap()
    w0l = nc.dram_tensor("w0l", [P, FC, DPAIR, 2, P], e4, kind="ExternalInput").ap()
    xh = nc.dram_tensor("xh", [P, NTB, DPAIR, 2, TBS], e4, kind="ExternalInput").ap()
    xl = nc.dram_tensor("xl", [P, NTB, DPAIR, 2, TBS], e4, kind="ExternalInput").ap()
    w3h = nc.dram_tensor("w3h", [P, FPAIR, 2, D], e4, kind="ExternalInput").ap()
    w3l = nc.dram_tensor("w3l", [P, FPAIR, 2, D], e4, kind="ExternalInput").ap()
    out = nc.dram_tensor("out", [T, D], f32, kind="ExternalOutput").ap()

    with tile.TileContext(nc) as tc:
        with (
            tc.tile_pool(name="weights", bufs=1) as wpool,
            tc.tile_pool(name="xin", bufs=x_bufs) as xpool,
            tc.tile_pool(name="gtiles", bufs=g_bufs) as gpool,
            tc.tile_pool(name="gf", bufs=gf_bufs) as gfpool,
            tc.tile_pool(name="ostage", bufs=o_sb_bufs) as opool,
        ):
            w0h_sb = wpool.tile([P, FC, DPAIR, 2, P], e4, name="w0h_sb", tag="w0h")
            w0l_sb = wpool.tile([P, FC, DPAIR, 2, P], e4, name="w0l_sb", tag="w0l")
            w3h_sb = wpool.tile([P, FPAIR, 2, D], e4, name="w3h_sb", tag="w3h")
            w3l_sb = wpool.tile([P, FPAIR, 2, D], e4, name="w3l_sb", tag="w3l")

            if warmup_mms:
                # separate scope: its PSUM bank is released before the main
                # PSUM pools open
                with (
                    tc.tile_pool(name="warm", bufs=1) as warmpool,
                    tc.tile_pool(name="warmps", bufs=1, space="PSUM") as warmpsum,
                ):
                    wsrc = warmpool.tile([P, 512], bf16, name="wsrc", tag="wsrc")
                    wps = warmpsum.tile([P, 512], f32, name="wps", tag="wps")
                    nc.gpsimd.memset(wsrc[:], 0.0)
                    for i in range(warmup_mms):
                        nc.tensor.matmul(wps[:], wsrc[:, :P], wsrc[:],
                                         start=(i == 0), stop=(i == warmup_mms - 1))

            # streamed x tiles, one pair per token block
            x_tiles = {}
            def load_x(tb):
                xht = xpool.tile([P, DPAIR, 2, TBS], e4, name=f"xh_{tb}", tag="xh")
                xlt = xpool.tile([P, DPAIR, 2, TBS], e4, name=f"xl_{tb}", tag="xl")
                nc.sync.dma_start(xht[:], xh[:, tb])
                nc.sync.dma_start(xlt[:], xl[:, tb])
                x_tiles[tb] = (xht, xlt)

            # DMA issue order = consumption order of the [A0|A1] B0 A2 B1 ...
            # software pipeline: x(tb0), x(tb1) (phases A0/A1 run interleaved
            # so each W0 chunk is consumed twice per arrival), all W0 chunks,
            # all W3 chunks (phase B0 runs after A0/A1), rest of x.
            load_x(0)
            load_x(1)
            for g in range(FC // wgrp):
                for sb_t, dr in ((w0h_sb, w0h), (w0l_sb, w0l)):
                    nc.sync.dma_start(sb_t[:, g * wgrp:(g + 1) * wgrp],
                                      dr[:, g * wgrp:(g + 1) * wgrp])
            for g in range(FPAIR // w3grp):
                for sb_t, dr in ((w3h_sb, w3h), (w3l_sb, w3l)):
                    nc.sync.dma_start(sb_t[:, g * w3grp:(g + 1) * w3grp],
                                      dr[:, g * w3grp:(g + 1) * w3grp])
            for tb in range(2, NTB):
                load_x(tb)

            with (
                tc.tile_pool(name="hps", bufs=h_bufs, space="PSUM") as hpsum,
                tc.tile_pool(name="ops", bufs=o_ps_bufs, space="PSUM") as opsum,
            ):
                g_pair = {}

                def start_a(tb):
                    g_hi = gpool.tile([P, FC, TBS], e4, name=f"ghi_{tb}", tag="ghi")
                    g_lo = gpool.tile([P, FC, TBS], e4, name=f"glo_{tb}", tag="glo")
                    g_pair[tb] = (g_hi, g_lo)
                    xht, xlt = x_tiles.pop(tb)
                    return (tb, g_hi, g_lo, xht, xlt)

                def emit_a_pack(ctx, fp):
                    # GEMM1: 3 fp8 terms, one PSUM group per fc; fcp fc-tiles
                    # share one PSUM pack so GELU + requant run as wide ops.
                    tb, g_hi, g_lo, xht, xlt = ctx
                    terms = ((w0h_sb, xht), (w0h_sb, xlt), (w0l_sb, xht))
                    nmm1 = len(terms) * DPAIR
                    h_ps = hpsum.tile([P, fcp, TBS], f32, name=f"h_{tb}_{fp}",
                                      tag="h")
                    for s in range(fcp):
                        fc = fp * fcp + s
                        k = 0
                        for wt, xt in terms:
                            for j in range(DPAIR):
                                nc.tensor.matmul(
                                    h_ps[:, s],
                                    wt[:, fc, j],
                                    xt[:, j],
                                    start=(k == 0),
                                    stop=(k == nmm1 - 1),
                                    perf_mode=PM.DoubleRow,
                                )
                                k += 1
                    gf = gfpool.tile([P, fcp, TBS], bf16, name=f"gf_{tb}_{fp}",
                                     tag="gf")
                    gslc = slice(fp * fcp, (fp + 1) * fcp)
                    nc.scalar.activation(gf[:], h_ps[:], AFT.Gelu,
                                         scale=1.0 / SW)
                    nc.scalar.activation(g_hi[:, gslc], h_ps[:], AFT.Gelu,
                                         scale=1.0 / SW)
                    nc.vector.scalar_tensor_tensor(g_lo[:, gslc],
                                                   g_hi[:, gslc], -1.0,
                                                   gf[:], op0=ALU.mult,
                                                   op1=ALU.add)

                def phase_a(tb):
                    ctx = start_a(tb)
                    for fp in range(FC // fcp):
                        emit_a_pack(ctx, fp)

                def phase_b(tb):
                    g_hi, g_lo = g_pair.pop(tb)
                    for ts in range(NTS):
                        for dc in range(ND):
                            o_ps = opsum.tile([P, DW], f32,
                                              name=f"o_{tb}_{ts}_{dc}", tag="o")
                            d0 = dc * DW
                            nmm = FPAIR * 3
                            k = 0
                            for jj in range(FPAIR):
                                gh = g_hi[:, 2 * jj:2 * jj + 2,
                                          ts * P:(ts + 1) * P]
                                gl = g_lo[:, 2 * jj:2 * jj + 2,
                                          ts * P:(ts + 1) * P]
                                for lhs, rhs in ((gh, w3h_sb), (gl, w3h_sb),
                                                 (gh, w3l_sb)):
                                    nc.tensor.matmul(
                                        o_ps[:],
                                        lhs,
                                        rhs[:, jj, :, d0:d0 + DW],
                                        start=(k == 0),
                                        stop=(k == nmm - 1),
                                        perf_mode=PM.DoubleRow,
                                    )
                                    k += 1
                            o_sb = opool.tile([P, DW], f32,
                                              name=f"os_{tb}_{ts}_{dc}", tag="os")
                            nc.vector.tensor_scalar_mul(o_sb[:], o_ps[:], 1.0 / SW)
                            nc.sync.dma_start(
                                out[tb * TBS + ts * P: tb * TBS + (ts + 1) * P,
                                    d0:d0 + DW],
                                o_sb[:],
                            )

                # A0 and A1 interleave per fc-pack: each W0 chunk is consumed
                # twice per DMA arrival, halving the startup weight-demand
                # rate below the DMA bus rate.
                ctx0 = start_a(0)
                ctx1 = start_a(1)
                for fp in range(FC // fcp):
                    emit_a_pack(ctx0, fp)
                    emit_a_pack(ctx1, fp)
                phase_b(0)
                for tb in range(2, NTB):
                    phase_a(tb)
                    phase_b(tb - 1)
                phase_b(NTB - 1)

    nc.compile()
    return nc


def _get_nc():
    if "nc" not in _cache:
        _cache["nc"] = _build_nc_fp8()
    return _cache["nc"]


def _make_cached_fn(nc):
    """Build a reusable jitted 8-core executable around bass2jax's bass_exec
    primitive (the same lowering run_bass_kernel_spmd uses under axon), so
    repeat kernel() calls skip retrace/relower."""
    import jax
    import numpy as np
    from jax.sharding import Mesh, PartitionSpec
    try:
        from jax.experimental.shard_map import shard_map
    except ImportError:
        from jax.shard_map import shard_map
    import concourse.mybir as mybir
    from concourse.bass2jax import (_bass_exec_p, install_neuronx_cc_hook,
                                    partition_id_tensor)

    install_neuronx_cc_hook()
    partition_name = nc.partition_id_tensor.name if nc.partition_id_tensor else None
    in_names, out_names, out_avals, zero_shapes = [], [], [], []
    for alloc in nc.m.functions[0].allocations:
        if not isinstance(alloc, mybir.MemoryLocationSet):
            continue
        name = alloc.memorylocations[0].name
        if alloc.kind == "ExternalInput":
            if name != partition_name:
                in_names.append(name)
        elif alloc.kind == "ExternalOutput":
            out_names.append(name)
            shape = tuple(alloc.tensor_shape)
            dtype = mybir.dt.np(alloc.dtype)
            out_avals.append(jax.core.ShapedArray(shape, dtype))
            zero_shapes.append((shape, dtype))
    n_params = len(in_names)
    all_in_names = list(in_names) + list(out_names)
    if partition_name is not None:
        all_in_names.append(partition_name)

    def _body(*args):
        ins = list(args[:n_params])
        outs = list(args[n_params:])
        extra = [partition_id_tensor()] if partition_name is not None else []
        return tuple(_bass_exec_p.bind(
            *ins, *outs, *extra,
            out_avals=tuple(out_avals),
            in_names=tuple(all_in_names),
            out_names=tuple(out_names),
            lowering_input_output_aliases=(),
            sim_require_finite=True,
            sim_require_nnan=True,
            nc=nc,
        ))

    devices = jax.devices()[:N_CORES]
    mesh = Mesh(np.asarray(devices), ("core",))
    fn = jax.jit(
        shard_map(_body, mesh=mesh,
                  in_specs=(PartitionSpec("core"),) * (n_params + len(out_names)),
                  out_specs=(PartitionSpec("core"),) * len(out_names),
                  check_rep=False),
        keep_unused=True)

    def run(in_maps):
        concat_in = [np.concatenate([np.asarray(m[n]) for m in in_maps], axis=0)
                     for n in in_names]
        concat_zeros = [np.zeros((N_CORES * s[0], *s[1:]), dt)
                        for s, dt in zero_shapes]
        outs = fn(*concat_in, *concat_zeros)
        return [
            {name: np.asarray(outs[i]).reshape(N_CORES, *out_avals[i].shape)[c]
             for i, name in enumerate(out_names)}
            for c in range(N_CORES)
        ]

    return run


def kernel(**inputs):
    import os
    import sys
    if "/opt/trn_rl_repo" not in sys.path:
        sys.path.insert(0, "/opt/trn_rl_repo")
    from concourse import bass_utils

    output_tensor = np.asarray(inputs["output_tensor"], dtype=np.float32)  # [1, 8]
    x = np.asarray(inputs["inputs"], dtype=np.float32)   # [1, 8, 2048, 1024]
    w0 = np.asarray(inputs["w0"], dtype=np.float32)      # [8, 4096, 1024]
    w3 = np.asarray(inputs["w3"], dtype=np.float32)      # [8, 1024, 4096]

    e4 = ml_dtypes.float8_e4m3

    def prep_expert(e):
        # hi/lo e4m3 decomposition; weights pre-scaled by SW so both parts
        # stay clear of the e4m3 subnormal floor (see module docstring).
        xe = x[0, e]
        xh8 = xe.astype(e4)
        xl8 = (xe - xh8.astype(np.float32)).astype(e4)
        w0s = w0[e] * np.float32(SW)
        w0h8 = w0s.astype(e4)
        w0l8 = (w0s - w0h8.astype(np.float32)).astype(e4)
        w3s = w3[e] * np.float32(SW)
        w3h8 = w3s.astype(e4)
        w3l8 = (w3s - w3h8.astype(np.float32)).astype(e4)

        def lay_x(a):      # [T, D] -> [P, NTB, DPAIR, 2, TBS]
            return np.ascontiguousarray(
                a.reshape(NTB, TBS, 2 * DPAIR, P).transpose(3, 0, 2, 1)
                .reshape(P, NTB, DPAIR, 2, TBS))

        def lay_w0(a):     # [F, D] -> [P, FC, DPAIR, 2, P]
            return np.ascontiguousarray(
                a.reshape(FC, P, 2 * DPAIR, P).transpose(3, 0, 2, 1)
                .reshape(P, FC, DPAIR, 2, P))

        def lay_w3(a):     # [D, F] -> [P, FPAIR, 2, D]
            return np.ascontiguousarray(
                a.T.reshape(2 * FPAIR, P, D).transpose(1, 0, 2)
                .reshape(P, FPAIR, 2, D))

        return {
            "xh": lay_x(xh8), "xl": lay_x(xl8),
            "w0h": lay_w0(w0h8), "w0l": lay_w0(w0l8),
            "w3h": lay_w3(w3h8), "w3l": lay_w3(w3l8),
        }

    from concurrent.futures import ThreadPoolExecutor
    with ThreadPoolExecutor(max_workers=N_CORES) as pool:
        in_maps = list(pool.map(prep_expert, range(N_CORES)))

    nc = _get_nc()
    results = None
    if "fast_fn" in _cache:
        try:
            results = _cache["fast_fn"](in_maps)
        except Exception:
            results = None
    if results is None:
        try:
            results = bass_utils.run_bass_kernel_spmd(
                nc, in_maps, core_ids=list(range(N_CORES))).results
        except ModuleNotFoundError:
            # trace path requested via env but axon NTFF hook missing
            os.environ["BASS_NEVER_TRACE"] = "1"
            results = bass_utils.run_bass_kernel_spmd(
                nc, in_maps, core_ids=list(range(N_CORES))).results
        try:
            fast = _make_cached_fn(nc)
            fast(in_maps)  # warm: jit trace + XLA/NEFF compile happens here
            _cache["fast_fn"] = fast
        except Exception:
            pass
    out_full = np.stack([results[e]["out"] for e in range(N_CORES)])[None]

    # unpopular experts with zero gating activity produce zeros
    unpop = output_tensor[:, NUM_LOCAL:].sum(axis=0) != 0
    mask = np.concatenate([np.ones(NUM_LOCAL, dtype=bool), unpop])
    out_full = out_full * mask[None, :, None, None].astype(np.float32)
    return out_full.astype(np.float32)


# revision 17
# speedup vs baseline: 1.3815x; 1.0442x over previous
"""Trainium2 Bass kernel for nn_Experts (grouped MoE expert MLP).

Computes, for each of 8 experts e:
    h   = x_e @ w0_e.T          # [2048,1024] @ [1024,4096] -> [2048,4096]
    g   = gelu_exact(h)
    out = g @ w3_e.T            # [2048,4096] @ [4096,1024] -> [2048,1024]
then masks unpopular experts with zero gating activity (output_tensor).

Sharding: expert-parallel, 1 expert per NeuronCore across 8 cores (SPMD —
one compiled NEFF, per-core input data).

Numerics/perf strategy: fp8 (e4m3) matmuls in DoubleRow perf mode (K=256 per
matmul, 0.5 cycles/row) with residual-corrected operands. Every GEMM is
evaluated as three fp8 term-GEMMs accumulated in ONE PSUM group:

    x @ W ~= X_hi @ W_hi + X_lo @ W_hi + X_hi @ W_lo

where X_hi = e4m3(x), X_lo = e4m3(x - X_hi) (the residual is representable
unscaled because x ~ N(0,1)), and W is pre-scaled by SW=64 on the host so
BOTH its hi part and its residual stay clear of e4m3's subnormal floor. All
three terms then share the same global scale (SW), so they can accumulate
into a single PSUM bank with no combine pass; the SW descale folds into the
GELU activation's input scale (GEMM1) or the output copy's scale (GEMM2).
Measured end-to-end rel err of this scheme: ~2.6e-3 (limit 2e-2).

g is re-quantized the same way: gelu writes G_hi = e4m3(g) and a bf16 copy
g_f; the DVE computes G_lo = e4m3(g_f - G_hi) in one scalar_tensor_tensor op.
"""

import numpy as np
import ml_dtypes

T = 2048      # tokens (capacity) per expert
D = 1024      # hidden
F = 4096      # ffn
P = 128       # partitions
TBS = 256     # token block (GEMM1 moving free dim = 2*256 DR-packed)
NTB = T // TBS        # 8
DPAIR = D // (2 * P)  # 4  k-pairs in GEMM1 contraction
FPAIR = F // (2 * P)  # 16 k-pairs in GEMM2 contraction
FC = F // P           # 32 f-chunks (GEMM1 output tiles per token block)
DW = 256              # GEMM2 output free-dim chunk
ND = D // DW          # 4
NTS = TBS // P        # 2 t-subblocks per token block
SW = 64.0             # global weight pre-scale (power of 2)
NUM_LOCAL = 4
N_CORES = 8

_cache = {}


def _build_nc_fp8(
    g_bufs=3,        # generations of the per-block G_hi/G_lo tiles (phase A
                     # runs one token block ahead of phase B)
    x_bufs=4,        # generations of the streamed per-block x tiles
    gf_bufs=3,       # gelu scratch ring ([128, FCP, 256] bf16 packs)
    h_bufs=3,        # GEMM1 PSUM packs in flight ([128, FCP, 256] = 2 banks)
    o_ps_bufs=2,     # GEMM2 PSUM tiles in flight
    o_sb_bufs=5,     # output staging ring
    warmup_mms=7,    # scratch matmuls riding out the PE cold-clock window
                     # while the initial DMAs land
    wgrp=4,          # fc per W0 DMA chunk
    w3grp=4,         # jj per W3 DMA chunk
    fcp=4,           # fc tiles per PSUM pack / GELU activation (amortizes the
                     # per-instruction activation init cost; keeps the Act
                     # engine under the PE group rate in phase A)
):
    import sys
    if "/opt/trn_rl_repo" not in sys.path:
        sys.path.insert(0, "/opt/trn_rl_repo")
    import concourse.tile as tile
    import concourse.mybir as mybir
    from concourse import bacc

    bf16 = mybir.dt.bfloat16
    f32 = mybir.dt.float32
    e4 = mybir.dt.float8e4
    AFT = mybir.ActivationFunctionType
    PM = mybir.MatmulPerfMode
    ALU = mybir.AluOpType

    nc = bacc.Bacc(
        "TRN2",
        target_bir_lowering=False,
        debug=False,
        enable_asserts=True,
        num_devices=N_CORES,
        dynamic_dma_scratch_size=2048,
    )

    # DRAM layouts (host-prepared, all contraction-major and DMA-contiguous):
    #   w0*[p, fc, j, i, m] = W0[f=fc*128+m, d=(2j+i)*128+p]
    #   x* [p, tb, j, i, t] = X [token=tb*256+t, d=(2j+i)*128+p]
    #   w3*[p, jj, i, dd]   = W3[dd, f=(2jj+i)*128+p]
    w0h = nc.dram_tensor("w0h", [P, FC, DPAIR, 2, P], e4, kind="ExternalInput").ap()
    w0l = nc.dram_tensor("w0l", [P, FC, DPAIR, 2, P], e4, kind="ExternalInput").ap()
    xh = nc.dram_tensor("xh", [P, NTB, DPAIR, 2, TBS], e4, kind="ExternalInput").ap()
    xl = nc.dram_tensor("xl", [P, NTB, DPAIR, 2, TBS], e4, kind="ExternalInput").ap()
    w3h = nc.dram_tensor("w3h", [P, FPAIR, 2, D], e4, kind="ExternalInput").ap()
    w3l = nc.dram_tensor("w3l", [P, FPAIR, 2, D], e4, kind="ExternalInput").ap()
    out = nc.dram_tensor("out", [T, D], f32, kind="ExternalOutput").ap()

    with tile.TileContext(nc) as tc:
        with (
            tc.tile_pool(name="weights", bufs=1) as wpool,
            tc.tile_pool(name="xin", bufs=x_bufs) as xpool,
            tc.tile_pool(name="gtiles", bufs=g_bufs) as gpool,
            tc.tile_pool(name="gf", bufs=gf_bufs) as gfpool,
            tc.tile_pool(name="ostage", bufs=o_sb_bufs) as opool,
        ):
            w0h_sb = wpool.tile([P, FC, DPAIR, 2, P], e4, name="w0h_sb", tag="w0h")
            w0l_sb = wpool.tile([P, FC, DPAIR, 2, P], e4, name="w0l_sb", tag="w0l")
            w3h_sb = wpool.tile([P, FPAIR, 2, D], e4, name="w3h_sb", tag="w3h")
            w3l_sb = wpool.tile([P, FPAIR, 2, D], e4, name="w3l_sb", tag="w3l")

            if warmup_mms:
                # separate scope: its PSUM bank is released before the main
                # PSUM pools open
                with (
                    tc.tile_pool(name="warm", bufs=1) as warmpool,
                    tc.tile_pool(name="warmps", bufs=1, space="PSUM") as warmpsum,
                ):
                    wsrc = warmpool.tile([P, 512], bf16, name="wsrc", tag="wsrc")
                    wps = warmpsum.tile([P, 512], f32, name="wps", tag="wps")
                    nc.gpsimd.memset(wsrc[:], 0.0)
                    for i in range(warmup_mms):
                        nc.tensor.matmul(wps[:], wsrc[:, :P], wsrc[:],
                                         start=(i == 0), stop=(i == warmup_mms - 1))

            # streamed x tiles, one pair per token block
            x_tiles = {}
            def load_x(tb):
                xht = xpool.tile([P, DPAIR, 2, TBS], e4, name=f"xh_{tb}", tag="xh")
                xlt = xpool.tile([P, DPAIR, 2, TBS], e4, name=f"xl_{tb}", tag="xl")
                nc.sync.dma_start(xht[:], xh[:, tb])
                nc.sync.dma_start(xlt[:], xl[:, tb])
                x_tiles[tb] = (xht, xlt)

            # DMA issue order = consumption order of the [A0|A1] B0 A2 B1 ...
            # software pipeline: x(tb0), two small leading W0 chunks (so A0's
            # first packs can start ~4us in), x(tb1), the rest of W0 (phases
            # A0/A1 run interleaved so each W0 chunk is consumed twice per
            # arrival), all W3 chunks (phase B0 runs after A0/A1), rest of x.
            load_x(0)
            for sb_t, dr in ((w0h_sb, w0h), (w0l_sb, w0l)):
                nc.sync.dma_start(sb_t[:, 0:2], dr[:, 0:2])
            load_x(1)
            for sb_t, dr in ((w0h_sb, w0h), (w0l_sb, w0l)):
                nc.sync.dma_start(sb_t[:, 2:4], dr[:, 2:4])
            for g in range(1, FC // wgrp):
                for sb_t, dr in ((w0h_sb, w0h), (w0l_sb, w0l)):
                    nc.sync.dma_start(sb_t[:, g * wgrp:(g + 1) * wgrp],
                                      dr[:, g * wgrp:(g + 1) * wgrp])
            for g in range(FPAIR // w3grp):
                for sb_t, dr in ((w3h_sb, w3h), (w3l_sb, w3l)):
                    nc.sync.dma_start(sb_t[:, g * w3grp:(g + 1) * w3grp],
                                      dr[:, g * w3grp:(g + 1) * w3grp])
            for tb in range(2, NTB):
                load_x(tb)

            with (
                tc.tile_pool(name="hps", bufs=h_bufs, space="PSUM") as hpsum,
                tc.tile_pool(name="ops", bufs=o_ps_bufs, space="PSUM") as opsum,
            ):
                g_pair = {}

                def start_a(tb):
                    g_hi = gpool.tile([P, FC, TBS], e4, name=f"ghi_{tb}", tag="ghi")
                    g_lo = gpool.tile([P, FC, TBS], e4, name=f"glo_{tb}", tag="glo")
                    g_pair[tb] = (g_hi, g_lo)
                    xht, xlt = x_tiles.pop(tb)
                    return (tb, g_hi, g_lo, xht, xlt)

                def emit_a_pack(ctx, fp):
                    # GEMM1: 3 fp8 terms, one PSUM group per fc; fcp fc-tiles
                    # share one PSUM pack so GELU + requant run as wide ops.
                    tb, g_hi, g_lo, xht, xlt = ctx
                    terms = ((w0h_sb, xht), (w0h_sb, xlt), (w0l_sb, xht))
                    nmm1 = len(terms) * DPAIR
                    h_ps = hpsum.tile([P, fcp, TBS], f32, name=f"h_{tb}_{fp}",
                                      tag="h")
                    for s in range(fcp):
                        fc = fp * fcp + s
                        k = 0
                        for wt, xt in terms:
                            for j in range(DPAIR):
                                nc.tensor.matmul(
                                    h_ps[:, s],
                                    wt[:, fc, j],
                                    xt[:, j],
                                    start=(k == 0),
                                    stop=(k == nmm1 - 1),
                                    perf_mode=PM.DoubleRow,
                                )
                                k += 1
                    gf = gfpool.tile([P, fcp, TBS], bf16, name=f"gf_{tb}_{fp}",
                                     tag="gf")
                    gslc = slice(fp * fcp, (fp + 1) * fcp)
                    nc.scalar.activation(gf[:], h_ps[:], AFT.Gelu,
                                         scale=1.0 / SW)
                    nc.scalar.activation(g_hi[:, gslc], h_ps[:], AFT.Gelu,
                                         scale=1.0 / SW)
                    nc.vector.scalar_tensor_tensor(g_lo[:, gslc],
                                                   g_hi[:, gslc], -1.0,
                                                   gf[:], op0=ALU.mult,
                                                   op1=ALU.add)

                def phase_a(tb):
                    ctx = start_a(tb)
                    for fp in range(FC // fcp):
                        emit_a_pack(ctx, fp)

                # The last CORR_DROP f-pair blocks of the two GEMM2 correction
                # terms are skipped: the residual error they'd remove is
                # ~sqrt(CORR_DROP/16) of each tensor's fp8 noise (~0.7%/block)
                # and the total stays at ~1.5e-2 vs the 2e-2 limit, while each
                # dropped block saves 64 matmuls.
                CORR_DROP = 2

                def emit_b_group(tb, g_hi, g_lo, ts, d0, dw, tag):
                    o_ps = opsum.tile([P, dw], f32, name=f"o_{tag}", tag="o")
                    mms = []
                    for jj in range(FPAIR):
                        gh = g_hi[:, 2 * jj:2 * jj + 2, ts * P:(ts + 1) * P]
                        gl = g_lo[:, 2 * jj:2 * jj + 2, ts * P:(ts + 1) * P]
                        mms.append((gh, w3h_sb, jj))
                        if jj < FPAIR - CORR_DROP:
                            mms.append((gl, w3h_sb, jj))
                            mms.append((gh, w3l_sb, jj))
                    for k, (lhs, rhs, jj) in enumerate(mms):
                        nc.tensor.matmul(
                            o_ps[:],
                            lhs,
                            rhs[:, jj, :, d0:d0 + dw],
                            start=(k == 0),
                            stop=(k == len(mms) - 1),
                            perf_mode=PM.DoubleRow,
                        )
                    o_sb = opool.tile([P, dw], f32, name=f"os_{tag}", tag="os")
                    nc.vector.tensor_scalar_mul(o_sb[:], o_ps[:], 1.0 / SW)
                    nc.sync.dma_start(
                        out[tb * TBS + ts * P: tb * TBS + (ts + 1) * P,
                            d0:d0 + dw],
                        o_sb[:],
                    )

                def phase_b(tb):
                    g_hi, g_lo = g_pair.pop(tb)
                    last_tb = tb == NTB - 1
                    for ts in range(NTS):
                        for dc in range(ND):
                            if last_tb and ts == NTS - 1 and dc == ND - 1:
                                # split the final tile so the tail chain
                                # (copy + DMA + drain) rides a smaller piece
                                for half in range(2):
                                    emit_b_group(tb, g_hi, g_lo, ts,
                                                 dc * DW + half * (DW // 2),
                                                 DW // 2, f"{tb}_{ts}_{dc}_{half}")
                            else:
                                emit_b_group(tb, g_hi, g_lo, ts, dc * DW, DW,
                                             f"{tb}_{ts}_{dc}")

                # A0 and A1 interleave per fc-pack: each W0 chunk is consumed
                # twice per DMA arrival, halving the startup weight-demand
                # rate below the DMA bus rate.
                ctx0 = start_a(0)
                ctx1 = start_a(1)
                for fp in range(FC // fcp):
                    emit_a_pack(ctx0, fp)
                    emit_a_pack(ctx1, fp)
                phase_b(0)
                for tb in range(2, NTB):
                    phase_a(tb)
                    phase_b(tb - 1)
                phase_b(NTB - 1)

    nc.compile()
    return nc


def _get_nc():
    if "nc" not in _cache:
        _cache["nc"] = _build_nc_fp8()
    return _cache["nc"]


def _make_cached_fn(nc):
    """Build a reusable jitted 8-core executable around bass2jax's bass_exec
    primitive (the same lowering run_bass_kernel_spmd uses under axon), so
    repeat kernel() calls skip retrace/relower."""
    import jax
    import numpy as np
    from jax.sharding import Mesh, PartitionSpec
    try:
        from jax.experimental.shard_map import shard_map
    except ImportError:
        from jax.shard_map import shard_map
    import concourse.mybir as mybir
    from concourse.bass2jax import (_bass_exec_p, install_neuronx_cc_hook,
                                    partition_id_tensor)

    install_neuronx_cc_hook()
    partition_name = nc.partition_id_tensor.name if nc.partition_id_tensor else None
    in_names, out_names, out_avals, zero_shapes = [], [], [], []
    for alloc in nc.m.functions[0].allocations:
        if not isinstance(alloc, mybir.MemoryLocationSet):
            continue
        name = alloc.memorylocations[0].name
        if alloc.kind == "ExternalInput":
            if name != partition_name:
                in_names.append(name)
        elif alloc.kind == "ExternalOutput":
            out_names.append(name)
            shape = tuple(alloc.tensor_shape)
            dtype = mybir.dt.np(alloc.dtype)
            out_avals.append(jax.core.ShapedArray(shape, dtype))
            zero_shapes.append((shape, dtype))
    n_params = len(in_names)
    all_in_names = list(in_names) + list(out_names)
    if partition_name is not None:
        all_in_names.append(partition_name)

    def _body(*args):
        ins = list(args[:n_params])
        outs = list(args[n_params:])
        extra = [partition_id_tensor()] if partition_name is not None else []
        return tuple(_bass_exec_p.bind(
            *ins, *outs, *extra,
            out_avals=tuple(out_avals),
            in_names=tuple(all_in_names),
            out_names=tuple(out_names),
            lowering_input_output_aliases=(),
            sim_require_finite=True,
            sim_require_nnan=True,
            nc=nc,
        ))

    devices = jax.devices()[:N_CORES]
    mesh = Mesh(np.asarray(devices), ("core",))
    fn = jax.jit(
        shard_map(_body, mesh=mesh,
                  in_specs=(PartitionSpec("core"),) * (n_params + len(out_names)),
                  out_specs=(PartitionSpec("core"),) * len(out_names),
                  check_rep=False),
        keep_unused=True)

    def run(in_maps):
        concat_in = [np.concatenate([np.asarray(m[n]) for m in in_maps], axis=0)
                     for n in in_names]
        concat_zeros = [np.zeros((N_CORES * s[0], *s[1:]), dt)
                        for s, dt in zero_shapes]
        outs = fn(*concat_in, *concat_zeros)
        return [
            {name: np.asarray(outs[i]).reshape(N_CORES, *out_avals[i].shape)[c]
             for i, name in enumerate(out_names)}
            for c in range(N_CORES)
        ]

    return run


def kernel(**inputs):
    import os
    import sys
    if "/opt/trn_rl_repo" not in sys.path:
        sys.path.insert(0, "/opt/trn_rl_repo")
    from concourse import bass_utils

    output_tensor = np.asarray(inputs["output_tensor"], dtype=np.float32)  # [1, 8]
    x = np.asarray(inputs["inputs"], dtype=np.float32)   # [1, 8, 2048, 1024]
    w0 = np.asarray(inputs["w0"], dtype=np.float32)      # [8, 4096, 1024]
    w3 = np.asarray(inputs["w3"], dtype=np.float32)      # [8, 1024, 4096]

    e4 = ml_dtypes.float8_e4m3

    def prep_expert(e):
        # hi/lo e4m3 decomposition; weights pre-scaled by SW so both parts
        # stay clear of the e4m3 subnormal floor (see module docstring).
        xe = x[0, e]
        xh8 = xe.astype(e4)
        xl8 = (xe - xh8.astype(np.float32)).astype(e4)
        w0s = w0[e] * np.float32(SW)
        w0h8 = w0s.astype(e4)
        w0l8 = (w0s - w0h8.astype(np.float32)).astype(e4)
        w3s = w3[e] * np.float32(SW)
        w3h8 = w3s.astype(e4)
        w3l8 = (w3s - w3h8.astype(np.float32)).astype(e4)

        def lay_x(a):      # [T, D] -> [P, NTB, DPAIR, 2, TBS]
            return np.ascontiguousarray(
                a.reshape(NTB, TBS, 2 * DPAIR, P).transpose(3, 0, 2, 1)
                .reshape(P, NTB, DPAIR, 2, TBS))

        def lay_w0(a):     # [F, D] -> [P, FC, DPAIR, 2, P]
            return np.ascontiguousarray(
                a.reshape(FC, P, 2 * DPAIR, P).transpose(3, 0, 2, 1)
                .reshape(P, FC, DPAIR, 2, P))

        def lay_w3(a):     # [D, F] -> [P, FPAIR, 2, D]
            return np.ascontiguousarray(
                a.T.reshape(2 * FPAIR, P, D).transpose(1, 0, 2)
                .reshape(P, FPAIR, 2, D))

        return {
            "xh": lay_x(xh8), "xl": lay_x(xl8),
            "w0h": lay_w0(w0h8), "w0l": lay_w0(w0l8),
            "w3h": lay_w3(w3h8), "w3l": lay_w3(w3l8),
        }

    from concurrent.futures import ThreadPoolExecutor
    with ThreadPoolExecutor(max_workers=N_CORES) as pool:
        in_maps = list(pool.map(prep_expert, range(N_CORES)))

    nc = _get_nc()
    results = None
    if "fast_fn" in _cache:
        try:
            results = _cache["fast_fn"](in_maps)
        except Exception:
            results = None
    if results is None:
        try:
            results = bass_utils.run_bass_kernel_spmd(
                nc, in_maps, core_ids=list(range(N_CORES))).results
        except ModuleNotFoundError:
            # trace path requested via env but axon NTFF hook missing
            os.environ["BASS_NEVER_TRACE"] = "1"
            results = bass_utils.run_bass_kernel_spmd(
                nc, in_maps, core_ids=list(range(N_CORES))).results
        try:
            fast = _make_cached_fn(nc)
            fast(in_maps)  # warm: jit trace + XLA/NEFF compile happens here
            _cache["fast_fn"] = fast
        except Exception:
            pass
    out_full = np.stack([results[e]["out"] for e in range(N_CORES)])[None]

    # unpopular experts with zero gating activity produce zeros
    unpop = output_tensor[:, NUM_LOCAL:].sum(axis=0) != 0
    mask = np.concatenate([np.ones(NUM_LOCAL, dtype=bool), unpop])
    out_full = out_full * mask[None, :, None, None].astype(np.float32)
    return out_full.astype(np.float32)


# revision 20
# speedup vs baseline: 1.3818x; 1.0002x over previous
"""Trainium2 Bass kernel for nn_Experts (grouped MoE expert MLP).

Computes, for each of 8 experts e:
    h   = x_e @ w0_e.T          # [2048,1024] @ [1024,4096] -> [2048,4096]
    g   = gelu_exact(h)
    out = g @ w3_e.T            # [2048,4096] @ [4096,1024] -> [2048,1024]
then masks unpopular experts with zero gating activity (output_tensor).

Sharding: expert-parallel, 1 expert per NeuronCore across 8 cores (SPMD —
one compiled NEFF, per-core input data).

Numerics/perf strategy: fp8 (e4m3) matmuls in DoubleRow perf mode (K=256 per
matmul, 0.5 cycles/row) with residual-corrected operands. Every GEMM is
evaluated as three fp8 term-GEMMs accumulated in ONE PSUM group:

    x @ W ~= X_hi @ W_hi + X_lo @ W_hi + X_hi @ W_lo

where X_hi = e4m3(x), X_lo = e4m3(x - X_hi) (the residual is representable
unscaled because x ~ N(0,1)), and W is pre-scaled by SW=64 on the host so
BOTH its hi part and its residual stay clear of e4m3's subnormal floor. All
three terms then share the same global scale (SW), so they can accumulate
into a single PSUM bank with no combine pass; the SW descale folds into the
GELU activation's input scale (GEMM1) or the output copy's scale (GEMM2).
Measured end-to-end rel err of this scheme: ~2.6e-3 (limit 2e-2).

g is re-quantized the same way: gelu writes G_hi = e4m3(g) and a bf16 copy
g_f; the DVE computes G_lo = e4m3(g_f - G_hi) in one scalar_tensor_tensor op.
"""

import numpy as np
import ml_dtypes

T = 2048      # tokens (capacity) per expert
D = 1024      # hidden
F = 4096      # ffn
P = 128       # partitions
TBS = 256     # token block (GEMM1 moving free dim = 2*256 DR-packed)
NTB = T // TBS        # 8
DPAIR = D // (2 * P)  # 4  k-pairs in GEMM1 contraction
FPAIR = F // (2 * P)  # 16 k-pairs in GEMM2 contraction
FC = F // P           # 32 f-chunks (GEMM1 output tiles per token block)
DW = 256              # GEMM2 output free-dim chunk
ND = D // DW          # 4
NTS = TBS // P        # 2 t-subblocks per token block
SW = 64.0             # global weight pre-scale (power of 2)
NUM_LOCAL = 4
N_CORES = 8

_cache = {}


def _build_nc_fp8(
    g_bufs=3,        # generations of the per-block G_hi/G_lo tiles (phase A
                     # runs one token block ahead of phase B)
    x_bufs=4,        # generations of the streamed per-block x tiles
    gf_bufs=3,       # gelu scratch ring ([128, FCP, 256] bf16 packs)
    h_bufs=3,        # GEMM1 PSUM packs in flight ([128, FCP, 256] = 2 banks)
    o_ps_bufs=2,     # GEMM2 PSUM tiles in flight
    o_sb_bufs=5,     # output staging ring
    warmup_mms=8,    # scratch matmuls riding out the PE cold-clock window
                     # while the initial DMAs land
    wgrp=4,          # fc per W0 DMA chunk
    w3grp=4,         # jj per W3 DMA chunk
    fcp=4,           # fc tiles per PSUM pack / GELU activation (amortizes the
                     # per-instruction activation init cost; keeps the Act
                     # engine under the PE group rate in phase A)
):
    import sys
    if "/opt/trn_rl_repo" not in sys.path:
        sys.path.insert(0, "/opt/trn_rl_repo")
    import concourse.tile as tile
    import concourse.mybir as mybir
    from concourse import bacc

    bf16 = mybir.dt.bfloat16
    f32 = mybir.dt.float32
    e4 = mybir.dt.float8e4
    AFT = mybir.ActivationFunctionType
    PM = mybir.MatmulPerfMode
    ALU = mybir.AluOpType

    nc = bacc.Bacc(
        "TRN2",
        target_bir_lowering=False,
        debug=False,
        enable_asserts=True,
        num_devices=N_CORES,
        dynamic_dma_scratch_size=2048,
    )

    # DRAM layouts (host-prepared, all contraction-major and DMA-contiguous):
    #   w0*[p, fc, j, i, m] = W0[f=fc*128+m, d=(2j+i)*128+p]
    #   x* [p, tb, j, i, t] = X [token=tb*256+t, d=(2j+i)*128+p]
    #   w3*[p, jj, i, dd]   = W3[dd, f=(2jj+i)*128+p]
    w0h = nc.dram_tensor("w0h", [P, FC, DPAIR, 2, P], e4, kind="ExternalInput").ap()
    w0l = nc.dram_tensor("w0l", [P, FC, DPAIR, 2, P], e4, kind="ExternalInput").ap()
    xh = nc.dram_tensor("xh", [P, NTB, DPAIR, 2, TBS], e4, kind="ExternalInput").ap()
    xl = nc.dram_tensor("xl", [P, NTB, DPAIR, 2, TBS], e4, kind="ExternalInput").ap()
    w3h = nc.dram_tensor("w3h", [P, FPAIR, 2, D], e4, kind="ExternalInput").ap()
    w3l = nc.dram_tensor("w3l", [P, FPAIR, 2, D], e4, kind="ExternalInput").ap()
    out = nc.dram_tensor("out", [T, D], f32, kind="ExternalOutput").ap()

    with tile.TileContext(nc) as tc:
        with (
            tc.tile_pool(name="weights", bufs=1) as wpool,
            tc.tile_pool(name="xin", bufs=x_bufs) as xpool,
            tc.tile_pool(name="gtiles", bufs=g_bufs) as gpool,
            tc.tile_pool(name="gf", bufs=gf_bufs) as gfpool,
            tc.tile_pool(name="ostage", bufs=o_sb_bufs) as opool,
        ):
            w0h_sb = wpool.tile([P, FC, DPAIR, 2, P], e4, name="w0h_sb", tag="w0h")
            w0l_sb = wpool.tile([P, FC, DPAIR, 2, P], e4, name="w0l_sb", tag="w0l")
            w3h_sb = wpool.tile([P, FPAIR, 2, D], e4, name="w3h_sb", tag="w3h")
            w3l_sb = wpool.tile([P, FPAIR, 2, D], e4, name="w3l_sb", tag="w3l")

            if warmup_mms:
                # separate scope: its PSUM bank is released before the main
                # PSUM pools open
                with (
                    tc.tile_pool(name="warm", bufs=1) as warmpool,
                    tc.tile_pool(name="warmps", bufs=1, space="PSUM") as warmpsum,
                ):
                    wsrc = warmpool.tile([P, 512], bf16, name="wsrc", tag="wsrc")
                    wps = warmpsum.tile([P, 512], f32, name="wps", tag="wps")
                    nc.gpsimd.memset(wsrc[:], 0.0)
                    for i in range(warmup_mms):
                        nc.tensor.matmul(wps[:], wsrc[:, :P], wsrc[:],
                                         start=(i == 0), stop=(i == warmup_mms - 1))

            # streamed x tiles, one pair per token block
            x_tiles = {}
            def load_x(tb):
                xht = xpool.tile([P, DPAIR, 2, TBS], e4, name=f"xh_{tb}", tag="xh")
                xlt = xpool.tile([P, DPAIR, 2, TBS], e4, name=f"xl_{tb}", tag="xl")
                nc.sync.dma_start(xht[:], xh[:, tb])
                nc.sync.dma_start(xlt[:], xl[:, tb])
                x_tiles[tb] = (xht, xlt)

            # DMA issue order = consumption order of the [A0|A1] B0 A2 B1 ...
            # software pipeline: x(tb0), two small leading W0 chunks (so A0's
            # first packs can start ~4us in), x(tb1), the rest of W0 (phases
            # A0/A1 run interleaved so each W0 chunk is consumed twice per
            # arrival), all W3 chunks (phase B0 runs after A0/A1), rest of x.
            load_x(0)
            for sb_t, dr in ((w0h_sb, w0h), (w0l_sb, w0l)):
                nc.sync.dma_start(sb_t[:, 0:2], dr[:, 0:2])
            for sb_t, dr in ((w0h_sb, w0h), (w0l_sb, w0l)):
                nc.sync.dma_start(sb_t[:, 2:4], dr[:, 2:4])
            load_x(1)
            for g in range(1, FC // wgrp):
                for sb_t, dr in ((w0h_sb, w0h), (w0l_sb, w0l)):
                    nc.sync.dma_start(sb_t[:, g * wgrp:(g + 1) * wgrp],
                                      dr[:, g * wgrp:(g + 1) * wgrp])
            # w3h fully before w3l: B groups consume the main (w3h) term first
            for sb_t, dr in ((w3h_sb, w3h), (w3l_sb, w3l)):
                for g in range(FPAIR // w3grp):
                    nc.sync.dma_start(sb_t[:, g * w3grp:(g + 1) * w3grp],
                                      dr[:, g * w3grp:(g + 1) * w3grp])
            for tb in range(2, NTB):
                load_x(tb)

            with (
                tc.tile_pool(name="hps", bufs=h_bufs, space="PSUM") as hpsum,
                tc.tile_pool(name="ops", bufs=o_ps_bufs, space="PSUM") as opsum,
            ):
                g_pair = {}

                def start_a(tb):
                    g_hi = gpool.tile([P, FC, TBS], e4, name=f"ghi_{tb}", tag="ghi")
                    g_lo = gpool.tile([P, FC, TBS], e4, name=f"glo_{tb}", tag="glo")
                    g_pair[tb] = (g_hi, g_lo)
                    xht, xlt = x_tiles.pop(tb)
                    return (tb, g_hi, g_lo, xht, xlt)

                def emit_a_pack(ctx, fp):
                    # GEMM1: 3 fp8 terms, one PSUM group per fc; fcp fc-tiles
                    # share one PSUM pack so GELU + requant run as wide ops.
                    tb, g_hi, g_lo, xht, xlt = ctx
                    terms = ((w0h_sb, xht), (w0h_sb, xlt), (w0l_sb, xht))
                    nmm1 = len(terms) * DPAIR
                    h_ps = hpsum.tile([P, fcp, TBS], f32, name=f"h_{tb}_{fp}",
                                      tag="h")
                    for s in range(fcp):
                        fc = fp * fcp + s
                        k = 0
                        for wt, xt in terms:
                            for j in range(DPAIR):
                                nc.tensor.matmul(
                                    h_ps[:, s],
                                    wt[:, fc, j],
                                    xt[:, j],
                                    start=(k == 0),
                                    stop=(k == nmm1 - 1),
                                    perf_mode=PM.DoubleRow,
                                )
                                k += 1
                    gf = gfpool.tile([P, fcp, TBS], bf16, name=f"gf_{tb}_{fp}",
                                     tag="gf")
                    gslc = slice(fp * fcp, (fp + 1) * fcp)
                    nc.scalar.activation(gf[:], h_ps[:], AFT.Gelu,
                                         scale=1.0 / SW)
                    nc.scalar.activation(g_hi[:, gslc], h_ps[:], AFT.Gelu,
                                         scale=1.0 / SW)
                    nc.vector.scalar_tensor_tensor(g_lo[:, gslc],
                                                   g_hi[:, gslc], -1.0,
                                                   gf[:], op0=ALU.mult,
                                                   op1=ALU.add)

                def phase_a(tb):
                    ctx = start_a(tb)
                    for fp in range(FC // fcp):
                        emit_a_pack(ctx, fp)

                # The last CORR_DROP f-pair blocks of the two GEMM2 correction
                # terms are skipped: the residual error they'd remove is
                # ~sqrt(CORR_DROP/16) of each tensor's fp8 noise (~0.7%/block)
                # and the total stays at ~1.5e-2 vs the 2e-2 limit, while each
                # dropped block saves 64 matmuls.
                CORR_DROP = 2

                def emit_b_group(tb, g_hi, g_lo, ts, d0, dw, tag):
                    o_ps = opsum.tile([P, dw], f32, name=f"o_{tag}", tag="o")
                    # term-major order (main w3h term first, w3l correction
                    # last) so the w3l chunks may arrive latest
                    mms = []
                    def gslice(t, jj):
                        return t[:, 2 * jj:2 * jj + 2, ts * P:(ts + 1) * P]
                    for jj in range(FPAIR):
                        mms.append((gslice(g_hi, jj), w3h_sb, jj))
                    for jj in range(FPAIR - CORR_DROP):
                        mms.append((gslice(g_lo, jj), w3h_sb, jj))
                    for jj in range(FPAIR - CORR_DROP):
                        mms.append((gslice(g_hi, jj), w3l_sb, jj))
                    for k, (lhs, rhs, jj) in enumerate(mms):
                        nc.tensor.matmul(
                            o_ps[:],
                            lhs,
                            rhs[:, jj, :, d0:d0 + dw],
                            start=(k == 0),
                            stop=(k == len(mms) - 1),
                            perf_mode=PM.DoubleRow,
                        )
                    o_sb = opool.tile([P, dw], f32, name=f"os_{tag}", tag="os")
                    nc.vector.tensor_scalar_mul(o_sb[:], o_ps[:], 1.0 / SW)
                    nc.sync.dma_start(
                        out[tb * TBS + ts * P: tb * TBS + (ts + 1) * P,
                            d0:d0 + dw],
                        o_sb[:],
                    )

                def phase_b(tb):
                    g_hi, g_lo = g_pair.pop(tb)
                    last_tb = tb == NTB - 1
                    for ts in range(NTS):
                        for dc in range(ND):
                            if last_tb and ts == NTS - 1 and dc == ND - 1:
                                # split the final tile so the tail chain
                                # (copy + DMA + drain) rides a smaller piece
                                for half in range(2):
                                    emit_b_group(tb, g_hi, g_lo, ts,
                                                 dc * DW + half * (DW // 2),
                                                 DW // 2, f"{tb}_{ts}_{dc}_{half}")
                            else:
                                emit_b_group(tb, g_hi, g_lo, ts, dc * DW, DW,
                                             f"{tb}_{ts}_{dc}")

                # A0 and A1 interleave per fc-pack: each W0 chunk is consumed
                # twice per DMA arrival, halving the startup weight-demand
                # rate below the DMA bus rate.
                ctx0 = start_a(0)
                ctx1 = start_a(1)
                for fp in range(FC // fcp):
                    emit_a_pack(ctx0, fp)
                    emit_a_pack(ctx1, fp)
                phase_b(0)
                for tb in range(2, NTB):
                    phase_a(tb)
                    phase_b(tb - 1)
                phase_b(NTB - 1)

    nc.compile()
    return nc


def _get_nc():
    if "nc" not in _cache:
        _cache["nc"] = _build_nc_fp8()
    return _cache["nc"]


def _make_cached_fn(nc):
    """Build a reusable jitted 8-core executable around bass2jax's bass_exec
    primitive (the same lowering run_bass_kernel_spmd uses under axon), so
    repeat kernel() calls skip retrace/relower."""
    import jax
    import numpy as np
    from jax.sharding import Mesh, PartitionSpec
    try:
        from jax.experimental.shard_map import shard_map
    except ImportError:
        from jax.shard_map import shard_map
    import concourse.mybir as mybir
    from concourse.bass2jax import (_bass_exec_p, install_neuronx_cc_hook,
                                    partition_id_tensor)

    install_neuronx_cc_hook()
    partition_name = nc.partition_id_tensor.name if nc.partition_id_tensor else None
    in_names, out_names, out_avals, zero_shapes = [], [], [], []
    for alloc in nc.m.functions[0].allocations:
        if not isinstance(alloc, mybir.MemoryLocationSet):
            continue
        name = alloc.memorylocations[0].name
        if alloc.kind == "ExternalInput":
            if name != partition_name:
                in_names.append(name)
        elif alloc.kind == "ExternalOutput":
            out_names.append(name)
            shape = tuple(alloc.tensor_shape)
            dtype = mybir.dt.np(alloc.dtype)
            out_avals.append(jax.core.ShapedArray(shape, dtype))
            zero_shapes.append((shape, dtype))
    n_params = len(in_names)
    all_in_names = list(in_names) + list(out_names)
    if partition_name is not None:
        all_in_names.append(partition_name)

    def _body(*args):
        ins = list(args[:n_params])
        outs = list(args[n_params:])
        extra = [partition_id_tensor()] if partition_name is not None else []
        return tuple(_bass_exec_p.bind(
            *ins, *outs, *extra,
            out_avals=tuple(out_avals),
            in_names=tuple(all_in_names),
            out_names=tuple(out_names),
            lowering_input_output_aliases=(),
            sim_require_finite=True,
            sim_require_nnan=True,
            nc=nc,
        ))

    devices = jax.devices()[:N_CORES]
    mesh = Mesh(np.asarray(devices), ("core",))
    fn = jax.jit(
        shard_map(_body, mesh=mesh,
                  in_specs=(PartitionSpec("core"),) * (n_params + len(out_names)),
                  out_specs=(PartitionSpec("core"),) * len(out_names),
                  check_rep=False),
        keep_unused=True)

    def run(in_maps):
        concat_in = [np.concatenate([np.asarray(m[n]) for m in in_maps], axis=0)
                     for n in in_names]
        concat_zeros = [np.zeros((N_CORES * s[0], *s[1:]), dt)
                        for s, dt in zero_shapes]
        outs = fn(*concat_in, *concat_zeros)
        return [
            {name: np.asarray(outs[i]).reshape(N_CORES, *out_avals[i].shape)[c]
             for i, name in enumerate(out_names)}
            for c in range(N_CORES)
        ]

    return run


def kernel(**inputs):
    import os
    import sys
    if "/opt/trn_rl_repo" not in sys.path:
        sys.path.insert(0, "/opt/trn_rl_repo")
    from concourse import bass_utils

    output_tensor = np.asarray(inputs["output_tensor"], dtype=np.float32)  # [1, 8]
    x = np.asarray(inputs["inputs"], dtype=np.float32)   # [1, 8, 2048, 1024]
    w0 = np.asarray(inputs["w0"], dtype=np.float32)      # [8, 4096, 1024]
    w3 = np.asarray(inputs["w3"], dtype=np.float32)      # [8, 1024, 4096]

    e4 = ml_dtypes.float8_e4m3

    def prep_expert(e):
        # hi/lo e4m3 decomposition; weights pre-scaled by SW so both parts
        # stay clear of the e4m3 subnormal floor (see module docstring).
        xe = x[0, e]
        xh8 = xe.astype(e4)
        xl8 = (xe - xh8.astype(np.float32)).astype(e4)
        w0s = w0[e] * np.float32(SW)
        w0h8 = w0s.astype(e4)
        w0l8 = (w0s - w0h8.astype(np.float32)).astype(e4)
        w3s = w3[e] * np.float32(SW)
        w3h8 = w3s.astype(e4)
        w3l8 = (w3s - w3h8.astype(np.float32)).astype(e4)

        def lay_x(a):      # [T, D] -> [P, NTB, DPAIR, 2, TBS]
            return np.ascontiguousarray(
                a.reshape(NTB, TBS, 2 * DPAIR, P).transpose(3, 0, 2, 1)
                .reshape(P, NTB, DPAIR, 2, TBS))

        def lay_w0(a):     # [F, D] -> [P, FC, DPAIR, 2, P]
            return np.ascontiguousarray(
                a.reshape(FC, P, 2 * DPAIR, P).transpose(3, 0, 2, 1)
                .reshape(P, FC, DPAIR, 2, P))

        def lay_w3(a):     # [D, F] -> [P, FPAIR, 2, D]
            return np.ascontiguousarray(
                a.T.reshape(2 * FPAIR, P, D).transpose(1, 0, 2)
                .reshape(P, FPAIR, 2, D))

        return {
            "xh": lay_x(xh8), "xl": lay_x(xl8),
            "w0h": lay_w0(w0h8), "w0l": lay_w0(w0l8),
            "w3h": lay_w3(w3h8), "w3l": lay_w3(w3l8),
        }

    from concurrent.futures import ThreadPoolExecutor
    with ThreadPoolExecutor(max_workers=N_CORES) as pool:
        in_maps = list(pool.map(prep_expert, range(N_CORES)))

    nc = _get_nc()
    results = None
    if "fast_fn" in _cache:
        try:
            results = _cache["fast_fn"](in_maps)
        except Exception:
            results = None
    if results is None:
        try:
            results = bass_utils.run_bass_kernel_spmd(
                nc, in_maps, core_ids=list(range(N_CORES))).results
        except ModuleNotFoundError:
            # trace path requested via env but axon NTFF hook missing
            os.environ["BASS_NEVER_TRACE"] = "1"
            results = bass_utils.run_bass_kernel_spmd(
                nc, in_maps, core_ids=list(range(N_CORES))).results
        try:
            fast = _make_cached_fn(nc)
            fast(in_maps)  # warm: jit trace + XLA/NEFF compile happens here
            _cache["fast_fn"] = fast
        except Exception:
            pass
    out_full = np.stack([results[e]["out"] for e in range(N_CORES)])[None]

    # unpopular experts with zero gating activity produce zeros
    unpop = output_tensor[:, NUM_LOCAL:].sum(axis=0) != 0
    mask = np.concatenate([np.ones(NUM_LOCAL, dtype=bool), unpop])
    out_full = out_full * mask[None, :, None, None].astype(np.float32)
    return out_full.astype(np.float32)


# revision 24
# speedup vs baseline: 1.3873x; 1.0040x over previous
"""Trainium2 Bass kernel for nn_Experts (grouped MoE expert MLP).

Computes, for each of 8 experts e:
    h   = x_e @ w0_e.T          # [2048,1024] @ [1024,4096] -> [2048,4096]
    g   = gelu_exact(h)
    out = g @ w3_e.T            # [2048,4096] @ [4096,1024] -> [2048,1024]
then masks unpopular experts with zero gating activity (output_tensor).

Sharding: expert-parallel, 1 expert per NeuronCore across 8 cores (SPMD —
one compiled NEFF, per-core input data).

Numerics/perf strategy: fp8 (e4m3) matmuls in DoubleRow perf mode (K=256 per
matmul, 0.5 cycles/row) with residual-corrected operands. Every GEMM is
evaluated as three fp8 term-GEMMs accumulated in ONE PSUM group:

    x @ W ~= X_hi @ W_hi + X_lo @ W_hi + X_hi @ W_lo

where X_hi = e4m3(x), X_lo = e4m3(x - X_hi) (the residual is representable
unscaled because x ~ N(0,1)), and W is pre-scaled by SW=64 on the host so
BOTH its hi part and its residual stay clear of e4m3's subnormal floor. All
three terms then share the same global scale (SW), so they can accumulate
into a single PSUM bank with no combine pass; the SW descale folds into the
GELU activation's input scale (GEMM1) or the output copy's scale (GEMM2).
Measured end-to-end rel err of this scheme: ~2.6e-3 (limit 2e-2).

g is re-quantized the same way: gelu writes G_hi = e4m3(g) and a bf16 copy
g_f; the DVE computes G_lo = e4m3(g_f - G_hi) in one scalar_tensor_tensor op.
"""

import numpy as np
import ml_dtypes

T = 2048      # tokens (capacity) per expert
D = 1024      # hidden
F = 4096      # ffn
P = 128       # partitions
TBS = 256     # token block (GEMM1 moving free dim = 2*256 DR-packed)
NTB = T // TBS        # 8
DPAIR = D // (2 * P)  # 4  k-pairs in GEMM1 contraction
FPAIR = F // (2 * P)  # 16 k-pairs in GEMM2 contraction
FC = F // P           # 32 f-chunks (GEMM1 output tiles per token block)
DW = 256              # GEMM2 output free-dim chunk
ND = D // DW          # 4
NTS = TBS // P        # 2 t-subblocks per token block
SW = 64.0             # global weight pre-scale (power of 2)
NUM_LOCAL = 4
N_CORES = 8
# The last CORR_DROP f-pair blocks of the two GEMM2 correction terms are
# skipped: the residual error they'd remove is ~sqrt(1/16) of each tensor's
# fp8 noise per block and the total stays ~1.5e-2 vs the 2e-2 limit, while
# each dropped block saves 64 matmuls (and w3l bytes never read).
CORR_DROP = 2

_cache = {}


def _build_nc_fp8(
    g_bufs=3,        # generations of the per-block G_hi/G_lo tiles (phase A
                     # runs one token block ahead of phase B)
    x_bufs=4,        # generations of the streamed per-block x tiles
    gf_bufs=3,       # gelu scratch ring ([128, FCP, 256] bf16 packs)
    h_bufs=3,        # GEMM1 PSUM packs in flight ([128, FCP, 256] = 2 banks)
    o_ps_bufs=2,     # GEMM2 PSUM tiles in flight
    o_sb_bufs=5,     # output staging ring
    warmup_mms=9,    # scratch matmuls riding out the PE cold-clock window
                     # while the initial DMAs land
    wgrp=4,          # fc per W0 DMA chunk
    w3grp=4,         # jj per W3 DMA chunk
    fcp=4,           # fc tiles per PSUM pack / GELU activation (amortizes the
                     # per-instruction activation init cost; keeps the Act
                     # engine under the PE group rate in phase A)
):
    import sys
    if "/opt/trn_rl_repo" not in sys.path:
        sys.path.insert(0, "/opt/trn_rl_repo")
    import concourse.tile as tile
    import concourse.mybir as mybir
    from concourse import bacc

    bf16 = mybir.dt.bfloat16
    f32 = mybir.dt.float32
    e4 = mybir.dt.float8e4
    AFT = mybir.ActivationFunctionType
    PM = mybir.MatmulPerfMode
    ALU = mybir.AluOpType

    nc = bacc.Bacc(
        "TRN2",
        target_bir_lowering=False,
        debug=False,
        enable_asserts=True,
        num_devices=N_CORES,
        dynamic_dma_scratch_size=2048,
    )

    # DRAM layouts (host-prepared, all contraction-major and DMA-contiguous):
    #   w0*[p, fc, j, i, m] = W0[f=fc*128+m, d=(2j+i)*128+p]
    #   x* [p, tb, j, i, t] = X [token=tb*256+t, d=(2j+i)*128+p]
    #   w3*[p, jj, i, dd]   = W3[dd, f=(2jj+i)*128+p]
    w0h = nc.dram_tensor("w0h", [P, FC, DPAIR, 2, P], e4, kind="ExternalInput").ap()
    w0l = nc.dram_tensor("w0l", [P, FC, DPAIR, 2, P], e4, kind="ExternalInput").ap()
    xh = nc.dram_tensor("xh", [P, NTB, DPAIR, 2, TBS], e4, kind="ExternalInput").ap()
    xl = nc.dram_tensor("xl", [P, NTB, DPAIR, 2, TBS], e4, kind="ExternalInput").ap()
    w3h = nc.dram_tensor("w3h", [P, FPAIR, 2, D], e4, kind="ExternalInput").ap()
    w3l = nc.dram_tensor("w3l", [P, FPAIR, 2, D], e4, kind="ExternalInput").ap()
    out = nc.dram_tensor("out", [T, D], f32, kind="ExternalOutput").ap()

    with tile.TileContext(nc) as tc:
        with (
            tc.tile_pool(name="weights", bufs=1) as wpool,
            tc.tile_pool(name="xin", bufs=x_bufs) as xpool,
            tc.tile_pool(name="gtiles", bufs=g_bufs) as gpool,
            tc.tile_pool(name="gf", bufs=gf_bufs) as gfpool,
            tc.tile_pool(name="ostage", bufs=o_sb_bufs) as opool,
        ):
            w0h_sb = wpool.tile([P, FC, DPAIR, 2, P], e4, name="w0h_sb", tag="w0h")
            w0l_sb = wpool.tile([P, FC, DPAIR, 2, P], e4, name="w0l_sb", tag="w0l")
            w3h_sb = wpool.tile([P, FPAIR, 2, D], e4, name="w3h_sb", tag="w3h")
            w3l_sb = wpool.tile([P, FPAIR, 2, D], e4, name="w3l_sb", tag="w3l")

            if warmup_mms:
                # separate scope: its PSUM bank is released before the main
                # PSUM pools open
                with (
                    tc.tile_pool(name="warm", bufs=1) as warmpool,
                    tc.tile_pool(name="warmps", bufs=1, space="PSUM") as warmpsum,
                ):
                    wsrc = warmpool.tile([P, 512], bf16, name="wsrc", tag="wsrc")
                    wps = warmpsum.tile([P, 512], f32, name="wps", tag="wps")
                    nc.gpsimd.memset(wsrc[:], 0.0)
                    for i in range(warmup_mms):
                        nc.tensor.matmul(wps[:], wsrc[:, :P], wsrc[:],
                                         start=(i == 0), stop=(i == warmup_mms - 1))

            # streamed x tiles, one pair per token block
            x_tiles = {}
            def load_x(tb):
                xht = xpool.tile([P, DPAIR, 2, TBS], e4, name=f"xh_{tb}", tag="xh")
                xlt = xpool.tile([P, DPAIR, 2, TBS], e4, name=f"xl_{tb}", tag="xl")
                nc.sync.dma_start(xht[:], xh[:, tb])
                nc.sync.dma_start(xlt[:], xl[:, tb])
                x_tiles[tb] = (xht, xlt)

            # DMA issue order = consumption order of the [A0|A1] B0 A2 B1 ...
            # software pipeline: x(tb0), two small leading W0 chunks (so A0's
            # first packs can start ~4us in), x(tb1), the rest of W0 (phases
            # A0/A1 run interleaved so each W0 chunk is consumed twice per
            # arrival), all W3 chunks (phase B0 runs after A0/A1), rest of x.
            load_x(0)
            for sb_t, dr in ((w0h_sb, w0h), (w0l_sb, w0l)):
                nc.sync.dma_start(sb_t[:, 0:2], dr[:, 0:2])
            for sb_t, dr in ((w0h_sb, w0h), (w0l_sb, w0l)):
                nc.sync.dma_start(sb_t[:, 2:4], dr[:, 2:4])
            load_x(1)
            for g in range(1, FC // wgrp):
                for sb_t, dr in ((w0h_sb, w0h), (w0l_sb, w0l)):
                    nc.sync.dma_start(sb_t[:, g * wgrp:(g + 1) * wgrp],
                                      dr[:, g * wgrp:(g + 1) * wgrp])
            # w3h fully before w3l: B groups consume the main (w3h) term first.
            # w3l's last CORR_DROP jj-pairs are never read (dropped correction
            # blocks) — don't transfer them.
            for g in range(FPAIR // w3grp):
                nc.sync.dma_start(w3h_sb[:, g * w3grp:(g + 1) * w3grp],
                                  w3h[:, g * w3grp:(g + 1) * w3grp])
            w3l_used = FPAIR - CORR_DROP
            for g in range((w3l_used + w3grp - 1) // w3grp):
                lo, hi = g * w3grp, min((g + 1) * w3grp, w3l_used)
                nc.sync.dma_start(w3l_sb[:, lo:hi], w3l[:, lo:hi])
            for tb in range(2, NTB):
                load_x(tb)

            with (
                tc.tile_pool(name="hps", bufs=h_bufs, space="PSUM") as hpsum,
                tc.tile_pool(name="ops", bufs=o_ps_bufs, space="PSUM") as opsum,
            ):
                g_pair = {}

                def start_a(tb):
                    g_hi = gpool.tile([P, FC, TBS], e4, name=f"ghi_{tb}", tag="ghi")
                    g_lo = gpool.tile([P, FC, TBS], e4, name=f"glo_{tb}", tag="glo")
                    g_pair[tb] = (g_hi, g_lo)
                    xht, xlt = x_tiles.pop(tb)
                    return (tb, g_hi, g_lo, xht, xlt)

                def emit_a_pack(ctx, fp):
                    # GEMM1: 3 fp8 terms, one PSUM group per fc; fcp fc-tiles
                    # share one PSUM pack so GELU + requant run as wide ops.
                    tb, g_hi, g_lo, xht, xlt = ctx
                    terms = ((w0h_sb, xht), (w0h_sb, xlt), (w0l_sb, xht))
                    nmm1 = len(terms) * DPAIR
                    h_ps = hpsum.tile([P, fcp, TBS], f32, name=f"h_{tb}_{fp}",
                                      tag="h")
                    for s in range(fcp):
                        fc = fp * fcp + s
                        k = 0
                        for wt, xt in terms:
                            for j in range(DPAIR):
                                nc.tensor.matmul(
                                    h_ps[:, s],
                                    wt[:, fc, j],
                                    xt[:, j],
                                    start=(k == 0),
                                    stop=(k == nmm1 - 1),
                                    perf_mode=PM.DoubleRow,
                                )
                                k += 1
                    gf = gfpool.tile([P, fcp, TBS], bf16, name=f"gf_{tb}_{fp}",
                                     tag="gf")
                    gslc = slice(fp * fcp, (fp + 1) * fcp)
                    nc.scalar.activation(gf[:], h_ps[:], AFT.Gelu,
                                         scale=1.0 / SW)
                    nc.scalar.activation(g_hi[:, gslc], h_ps[:], AFT.Gelu,
                                         scale=1.0 / SW)
                    nc.vector.scalar_tensor_tensor(g_lo[:, gslc],
                                                   g_hi[:, gslc], -1.0,
                                                   gf[:], op0=ALU.mult,
                                                   op1=ALU.add)

                def phase_a(tb):
                    ctx = start_a(tb)
                    for fp in range(FC // fcp):
                        emit_a_pack(ctx, fp)

                def emit_b_group(tb, g_hi, g_lo, ts, d0, dw, tag):
                    o_ps = opsum.tile([P, dw], f32, name=f"o_{tag}", tag="o")
                    # term-major order (main w3h term first, w3l correction
                    # last) so the w3l chunks may arrive latest
                    mms = []
                    def gslice(t, jj):
                        return t[:, 2 * jj:2 * jj + 2, ts * P:(ts + 1) * P]
                    for jj in range(FPAIR):
                        mms.append((gslice(g_hi, jj), w3h_sb, jj))
                    for jj in range(FPAIR - CORR_DROP):
                        mms.append((gslice(g_lo, jj), w3h_sb, jj))
                    for jj in range(FPAIR - CORR_DROP):
                        mms.append((gslice(g_hi, jj), w3l_sb, jj))
                    for k, (lhs, rhs, jj) in enumerate(mms):
                        nc.tensor.matmul(
                            o_ps[:],
                            lhs,
                            rhs[:, jj, :, d0:d0 + dw],
                            start=(k == 0),
                            stop=(k == len(mms) - 1),
                            perf_mode=PM.DoubleRow,
                        )
                    o_sb = opool.tile([P, dw], f32, name=f"os_{tag}", tag="os")
                    nc.vector.tensor_scalar_mul(o_sb[:], o_ps[:], 1.0 / SW)
                    nc.sync.dma_start(
                        out[tb * TBS + ts * P: tb * TBS + (ts + 1) * P,
                            d0:d0 + dw],
                        o_sb[:],
                    )

                def phase_b(tb):
                    g_hi, g_lo = g_pair.pop(tb)
                    last_tb = tb == NTB - 1
                    for ts in range(NTS):
                        for dc in range(ND):
                            if last_tb and ts == NTS - 1 and dc == ND - 1:
                                # split the final tile so the tail chain
                                # (copy + DMA + drain) rides a smaller piece
                                for half in range(2):
                                    emit_b_group(tb, g_hi, g_lo, ts,
                                                 dc * DW + half * (DW // 2),
                                                 DW // 2, f"{tb}_{ts}_{dc}_{half}")
                            else:
                                emit_b_group(tb, g_hi, g_lo, ts, dc * DW, DW,
                                             f"{tb}_{ts}_{dc}")

                # A0 and A1 interleave per fc-pack: each W0 chunk is consumed
                # twice per DMA arrival, halving the startup weight-demand
                # rate below the DMA bus rate.
                ctx0 = start_a(0)
                ctx1 = start_a(1)
                for fp in range(FC // fcp):
                    emit_a_pack(ctx0, fp)
                    emit_a_pack(ctx1, fp)
                phase_b(0)
                for tb in range(2, NTB):
                    phase_a(tb)
                    phase_b(tb - 1)
                phase_b(NTB - 1)

    nc.compile()
    return nc


def _get_nc():
    if "nc" not in _cache:
        _cache["nc"] = _build_nc_fp8()
    return _cache["nc"]


def _make_cached_fn(nc):
    """Build a reusable jitted 8-core executable around bass2jax's bass_exec
    primitive (the same lowering run_bass_kernel_spmd uses under axon), so
    repeat kernel() calls skip retrace/relower."""
    import jax
    import numpy as np
    from jax.sharding import Mesh, PartitionSpec
    try:
        from jax.experimental.shard_map import shard_map
    except ImportError:
        from jax.shard_map import shard_map
    import concourse.mybir as mybir
    from concourse.bass2jax import (_bass_exec_p, install_neuronx_cc_hook,
                                    partition_id_tensor)

    install_neuronx_cc_hook()
    partition_name = nc.partition_id_tensor.name if nc.partition_id_tensor else None
    in_names, out_names, out_avals, zero_shapes = [], [], [], []
    for alloc in nc.m.functions[0].allocations:
        if not isinstance(alloc, mybir.MemoryLocationSet):
            continue
        name = alloc.memorylocations[0].name
        if alloc.kind == "ExternalInput":
            if name != partition_name:
                in_names.append(name)
        elif alloc.kind == "ExternalOutput":
            out_names.append(name)
            shape = tuple(alloc.tensor_shape)
            dtype = mybir.dt.np(alloc.dtype)
            out_avals.append(jax.core.ShapedArray(shape, dtype))
            zero_shapes.append((shape, dtype))
    n_params = len(in_names)
    all_in_names = list(in_names) + list(out_names)
    if partition_name is not None:
        all_in_names.append(partition_name)

    def _body(*args):
        ins = list(args[:n_params])
        outs = list(args[n_params:])
        extra = [partition_id_tensor()] if partition_name is not None else []
        return tuple(_bass_exec_p.bind(
            *ins, *outs, *extra,
            out_avals=tuple(out_avals),
            in_names=tuple(all_in_names),
            out_names=tuple(out_names),
            lowering_input_output_aliases=(),
            sim_require_finite=True,
            sim_require_nnan=True,
            nc=nc,
        ))

    devices = jax.devices()[:N_CORES]
    mesh = Mesh(np.asarray(devices), ("core",))
    fn = jax.jit(
        shard_map(_body, mesh=mesh,
                  in_specs=(PartitionSpec("core"),) * (n_params + len(out_names)),
                  out_specs=(PartitionSpec("core"),) * len(out_names),
                  check_rep=False),
        keep_unused=True)

    def run(in_maps):
        concat_in = [np.concatenate([np.asarray(m[n]) for m in in_maps], axis=0)
                     for n in in_names]
        concat_zeros = [np.zeros((N_CORES * s[0], *s[1:]), dt)
                        for s, dt in zero_shapes]
        outs = fn(*concat_in, *concat_zeros)
        return [
            {name: np.asarray(outs[i]).reshape(N_CORES, *out_avals[i].shape)[c]
             for i, name in enumerate(out_names)}
            for c in range(N_CORES)
        ]

    return run


def kernel(**inputs):
    import os
    import sys
    if "/opt/trn_rl_repo" not in sys.path:
        sys.path.insert(0, "/opt/trn_rl_repo")
    from concourse import bass_utils

    output_tensor = np.asarray(inputs["output_tensor"], dtype=np.float32)  # [1, 8]
    x = np.asarray(inputs["inputs"], dtype=np.float32)   # [1, 8, 2048, 1024]
    w0 = np.asarray(inputs["w0"], dtype=np.float32)      # [8, 4096, 1024]
    w3 = np.asarray(inputs["w3"], dtype=np.float32)      # [8, 1024, 4096]

    e4 = ml_dtypes.float8_e4m3

    def prep_expert(e):
        # hi/lo e4m3 decomposition; weights pre-scaled by SW so both parts
        # stay clear of the e4m3 subnormal floor (see module docstring).
        xe = x[0, e]
        xh8 = xe.astype(e4)
        xl8 = (xe - xh8.astype(np.float32)).astype(e4)
        w0s = w0[e] * np.float32(SW)
        w0h8 = w0s.astype(e4)
        w0l8 = (w0s - w0h8.astype(np.float32)).astype(e4)
        w3s = w3[e] * np.float32(SW)
        w3h8 = w3s.astype(e4)
        w3l8 = (w3s - w3h8.astype(np.float32)).astype(e4)

        def lay_x(a):      # [T, D] -> [P, NTB, DPAIR, 2, TBS]
            return np.ascontiguousarray(
                a.reshape(NTB, TBS, 2 * DPAIR, P).transpose(3, 0, 2, 1)
                .reshape(P, NTB, DPAIR, 2, TBS))

        def lay_w0(a):     # [F, D] -> [P, FC, DPAIR, 2, P]
            return np.ascontiguousarray(
                a.reshape(FC, P, 2 * DPAIR, P).transpose(3, 0, 2, 1)
                .reshape(P, FC, DPAIR, 2, P))

        def lay_w3(a):     # [D, F] -> [P, FPAIR, 2, D]
            return np.ascontiguousarray(
                a.T.reshape(2 * FPAIR, P, D).transpose(1, 0, 2)
                .reshape(P, FPAIR, 2, D))

        return {
            "xh": lay_x(xh8), "xl": lay_x(xl8),
            "w0h": lay_w0(w0h8), "w0l": lay_w0(w0l8),
            "w3h": lay_w3(w3h8), "w3l": lay_w3(w3l8),
        }

    from concurrent.futures import ThreadPoolExecutor
    with ThreadPoolExecutor(max_workers=N_CORES) as pool:
        in_maps = list(pool.map(prep_expert, range(N_CORES)))

    nc = _get_nc()
    results = None
    if "fast_fn" in _cache:
        try:
            results = _cache["fast_fn"](in_maps)
        except Exception:
            results = None
    if results is None:
        try:
            results = bass_utils.run_bass_kernel_spmd(
                nc, in_maps, core_ids=list(range(N_CORES))).results
        except ModuleNotFoundError:
            # trace path requested via env but axon NTFF hook missing
            os.environ["BASS_NEVER_TRACE"] = "1"
            results = bass_utils.run_bass_kernel_spmd(
                nc, in_maps, core_ids=list(range(N_CORES))).results
        try:
            fast = _make_cached_fn(nc)
            fast(in_maps)  # warm: jit trace + XLA/NEFF compile happens here
            _cache["fast_fn"] = fast
        except Exception:
            pass
    out_full = np.stack([results[e]["out"] for e in range(N_CORES)])[None]

    # unpopular experts with zero gating activity produce zeros
    unpop = output_tensor[:, NUM_LOCAL:].sum(axis=0) != 0
    mask = np.concatenate([np.ones(NUM_LOCAL, dtype=bool), unpop])
    out_full = out_full * mask[None, :, None, None].astype(np.float32)
    return out_full.astype(np.float32)
